# revision 1
# baseline (speedup 1.0000x reference)
"""Trainium2 Bass kernel for segment-attention pooling (EquivariantPooling).

Math (per reference):
  g = batch_softmax(tanh(x@gw1+gb1)@gw2+gb2);  global_pool = segsum(x*g)
  l = batch_softmax(mask(tanh(x@lw1+lb1)@lw2+lb2));  lys_pool = segsum(x*l)
  out = (concat(global_pool, lys_pool)/sqrt(n_seg)) @ ow + ob

Strategy: batch ids are sorted -> contiguous segments. Host splits the 1024
segments into 8 groups of 128 (one per core), pads every segment to a common
length L (multiple of 128) and uploads a pre-tiled bf16 copy of x.  The device
program is identical on all cores (SPMD); only input data differs.
"""

import math
import os

import numpy as np
import ml_dtypes

import concourse.bass as bass
import concourse.tile as tile
import concourse.mybir as mybir
from concourse import bacc
from concourse.alu_op_type import AluOpType
from concourse.bass_utils import run_bass_kernel_spmd

F32 = mybir.dt.float32
F32R = mybir.dt.float32r
BF16 = mybir.dt.bfloat16
AX = mybir.AxisListType.X
AF = mybir.ActivationFunctionType

N_CORES = 8
H = 256
HH = 128  # hidden dim of attention MLPs
NEG = -1.0e9

_cache = {}


def _build(L, segs_per_core):
    """Build the per-core Bass program. Returns (nc, names)."""
    ablate = set(os.environ.get("KERNEL_ABLATE", "").split(","))
    TPS = L // 128            # tiles per segment
    CH_SEGS = 8               # segments per chunk
    NCHUNK = segs_per_core // CH_SEGS
    TILES_CH = CH_SEGS * TPS  # tiles per chunk
    NODES_CH = 128 * TILES_CH
    GT = 4                    # tiles per L1 group (512 nodes)
    NGROUP = TILES_CH // GT
    SEGS = segs_per_core

    nc = bacc.Bacc("TRN2", target_bir_lowering=False, debug=False,
                   num_devices=N_CORES)

    x_d = nc.dram_tensor("x", [NCHUNK, 128, TILES_CH * H], BF16,
                         kind="ExternalInput").ap()
    a_d = nc.dram_tensor("amask", [128, 2 * SEGS * TPS], F32,
                         kind="ExternalInput").ap()
    rsn_d = nc.dram_tensor("rsn", [2 * CH_SEGS, NCHUNK], F32,
                           kind="ExternalInput").ap()
    w1_d = nc.dram_tensor("w1", [128, 512], BF16, kind="ExternalInput").ap()
    b1_d = nc.dram_tensor("b1", [128, 2], F32, kind="ExternalInput").ap()
    gl2_d = nc.dram_tensor("gl2", [128, 2], BF16, kind="ExternalInput").ap()
    ow_d = nc.dram_tensor("ow", [128, 4 * H], F32, kind="ExternalInput").ap()
    ob_d = nc.dram_tensor("ob", [1, H], F32, kind="ExternalInput").ap()
    ones_d = nc.dram_tensor("ones", [1, 128], F32, kind="ExternalInput").ap()
    idn_d = nc.dram_tensor("idn", [128, 128], BF16, kind="ExternalInput").ap()
    idnf_d = nc.dram_tensor("idnf", [128, 128], F32, kind="ExternalInput").ap()
    y_d = nc.dram_tensor("y", [SEGS, H], F32, kind="ExternalOutput").ap()

    with tile.TileContext(nc) as tc:
        with (
            tc.tile_pool(name="const", bufs=1) as cpool,
            tc.tile_pool(name="nat", bufs=2) as nat_pool,
            tc.tile_pool(name="xT", bufs=2) as xT_pool,
            tc.tile_pool(name="h", bufs=2) as h_pool,
            tc.tile_pool(name="dE", bufs=2) as dE_pool,
            tc.tile_pool(name="small", bufs=3) as sm_pool,
            tc.tile_pool(name="acc", bufs=1) as acc_pool,
            tc.tile_pool(name="tp", bufs=2, space="PSUM") as tpsum,
            tc.tile_pool(name="hp", bufs=1, space="PSUM") as hpsum,
            tc.tile_pool(name="sp", bufs=1, space="PSUM") as spsum,
            tc.tile_pool(name="pp", bufs=1, space="PSUM") as ppsum,
            tc.tile_pool(name="st", bufs=2, space="PSUM") as stpsum,
        ):
            # ---- constants ----
            A_sb = cpool.tile([128, 2 * SEGS * TPS], F32, tag="A")
            nc.sync.dma_start(A_sb[:], a_d[:])
            RSN_sb = cpool.tile([2 * CH_SEGS, NCHUNK], F32, tag="RSN")
            nc.sync.dma_start(RSN_sb[:], rsn_d[:])
            W1_sb = cpool.tile([128, 512], BF16, tag="W1")
            nc.sync.dma_start(W1_sb[:], w1_d[:])
            B1_sb = cpool.tile([128, 2], F32, tag="B1")
            nc.sync.dma_start(B1_sb[:], b1_d[:])
            GL2_sb = cpool.tile([128, 2], BF16, tag="GL2")
            nc.sync.dma_start(GL2_sb[:], gl2_d[:])
            OW_sb = cpool.tile([128, 4 * H], F32, tag="OW")
            nc.sync.dma_start(OW_sb[:], ow_d[:])
            OB_sb = cpool.tile([1, H], F32, tag="OB")
            nc.sync.dma_start(OB_sb[:], ob_d[:])
            ONES_sb = cpool.tile([1, 128], F32, tag="ONES")
            nc.sync.dma_start(ONES_sb[:], ones_d[:])
            IDN_sb = cpool.tile([128, 128], BF16, tag="IDN")
            nc.sync.dma_start(IDN_sb[:], idn_d[:])
            IDNF_sb = cpool.tile([128, 128], F32, tag="IDNF")
            nc.sync.dma_start(IDNF_sb[:], idnf_d[:])

            # pooled^T accumulator: cols (a*2+h)*SEGS + seg
            pTsb = acc_pool.tile([128, 4 * SEGS], F32, tag="pT")

            for c in range(NCHUNK):
                # ---- load chunk (pre-tiled bf16) ----
                nat = nat_pool.tile([128, TILES_CH * H], BF16, tag="nat")
                nc.sync.dma_start(nat[:], x_d[c])

                # ---- transpose x -> xT  (cols: k*NODES_CH + n_local) ----
                xT = xT_pool.tile([128, 2 * NODES_CH], BF16, tag="xT")
                nb = 0
                for k in range(2 if "notr" not in ablate else 0):
                    for b in range(TILES_CH // 4):
                        tp = tpsum.tile([128, 512], BF16, tag="tp")
                        for j in range(4):
                            t = b * 4 + j
                            nc.tensor.transpose(
                                tp[:, 128 * j:128 * (j + 1)],
                                nat[:, t * H + 128 * k: t * H + 128 * k + 128],
                                IDN_sb[:])
                        dst = xT[:, k * NODES_CH + 512 * b:
                                 k * NODES_CH + 512 * (b + 1)]
                        if nb % 2 == 0:
                            nc.vector.tensor_copy(dst, tp[:])
                        else:
                            nc.scalar.copy(dst, tp[:])
                        nb += 1

                # ---- L1 + tanh + L2 ----
                s_ps = spsum.tile([128, 2 * TILES_CH], F32, tag="s")
                for g in range(NGROUP if "nomlp" not in ablate else 0):
                    for a in range(2):
                        hp = hpsum.tile([128, 512], F32, tag=f"hp{a}")
                        for k in range(2):
                            nc.tensor.matmul(
                                hp[:],
                                lhsT=W1_sb[:, (a * 2 + k) * 128:
                                           (a * 2 + k + 1) * 128],
                                rhs=xT[:, k * NODES_CH + 512 * g:
                                       k * NODES_CH + 512 * (g + 1)],
                                start=(k == 0), stop=(k == 1))
                        hsb = h_pool.tile([128, 512], BF16, tag=f"h{a}")
                        nc.scalar.activation(hsb[:], hp[:], AF.Tanh,
                                             bias=B1_sb[:, a:a + 1])
                        for j in range(GT if "nol2" not in ablate else 0):
                            t = g * GT + j
                            nc.tensor.matmul(
                                s_ps[:, 2 * t + a: 2 * t + a + 1],
                                lhsT=hsb[:, 128 * j:128 * (j + 1)],
                                rhs=GL2_sb[:, a:a + 1],
                                start=True, stop=True)

                # ---- segment softmax stats ----
                if "nostats" in ablate:
                    Ew = dE_pool.tile([128, 2 * TILES_CH], BF16, tag="Ew")
                    nc.vector.memset(Ew[:], 0.0)
                    stats_rng = []
                else:
                    stats_rng = [0]
                for _ in stats_rng:
                    # d = s + A  (A holds -1e9 pad/lysine masks, gb2/lb2)
                    d = dE_pool.tile([128, 2 * TILES_CH], F32, tag="d")
                    nc.vector.tensor_tensor(d[:], s_ps[:],
                                            A_sb[:, c * 2 * TILES_CH:
                                                 (c + 1) * 2 * TILES_CH],
                                            AluOpType.add)
                    d4 = d[:].rearrange("p (s r a) -> p s a r",
                                        s=CH_SEGS, r=TPS, a=2)
                    M1 = sm_pool.tile([128, 16], F32, tag="M1")
                    nc.vector.tensor_reduce(
                        M1[:].rearrange("p (s a) -> p s a", a=2), d4,
                        axis=AX, op=AluOpType.max)
                    # cross-partition reduce via PE transpose
                    st = stpsum.tile([128, 160], F32, tag="st")
                    nc.tensor.matmul(st[0:16, 0:128],
                                     lhsT=M1[:],
                                     rhs=IDNF_sb[:], is_transpose=True,
                                     start=True, stop=True)
                    mcol = sm_pool.tile([16, 1], F32, tag="mcol")
                    nc.vector.tensor_reduce(mcol[:], st[0:16, 0:128], axis=AX,
                                            op=AluOpType.max)
                    nc.vector.tensor_scalar_max(mcol[:], mcol[:], 0.0)
                    nc.tensor.matmul(st[0:1, 144:160],
                                     lhsT=mcol[:],
                                     rhs=IDNF_sb[0:16, 0:16], is_transpose=True,
                                     start=True, stop=True)
                    mrow = sm_pool.tile([1, 16], F32, tag="mrow")
                    nc.vector.tensor_copy(mrow[:], st[0:1, 144:160])
                    nc.tensor.matmul(st[:, 128:144],
                                     lhsT=ONES_sb[:],
                                     rhs=mrow[:],
                                     start=True, stop=True)
                    mrep = st[:, 128:144].rearrange("p (s a) -> p s a", a=2) \
                        .unsqueeze(2).broadcast_to([128, CH_SEGS, TPS, 2])
                    d2 = dE_pool.tile([128, 2 * TILES_CH], F32, tag="d2")
                    d24 = d2[:].rearrange("p (s r a) -> p s r a",
                                          s=CH_SEGS, r=TPS, a=2)
                    nc.vector.tensor_tensor(
                        d24, d[:].rearrange("p (s r a) -> p s r a",
                                            s=CH_SEGS, r=TPS, a=2),
                        mrep, AluOpType.subtract)
                    # e = exp(d2)
                    E = dE_pool.tile([128, 2 * TILES_CH], BF16, tag="E")
                    nc.scalar.activation(E[:], d2[:], AF.Exp)
                    Z1 = sm_pool.tile([128, 16], F32, tag="Z1")
                    nc.vector.tensor_reduce(
                        Z1[:].rearrange("p (s a) -> p s a", a=2),
                        E[:].rearrange("p (s r a) -> p s a r",
                                       s=CH_SEGS, r=TPS, a=2),
                        axis=AX, op=AluOpType.add)
                    st2 = stpsum.tile([128, 160], F32, tag="st")
                    nc.tensor.matmul(st2[0:16, 0:128],
                                     lhsT=Z1[:],
                                     rhs=IDNF_sb[:], is_transpose=True,
                                     start=True, stop=True)
                    zcol = sm_pool.tile([16, 1], F32, tag="zcol")
                    nc.vector.tensor_reduce(zcol[:], st2[0:16, 0:128], axis=AX,
                                            op=AluOpType.add)
                    nc.vector.tensor_scalar_add(zcol[:], zcol[:], 1.0e-8)
                    zinv = sm_pool.tile([16, 1], F32, tag="zinv")
                    nc.vector.reciprocal(zinv[:], zcol[:])
                    sc = sm_pool.tile([16, 1], F32, tag="sc")
                    nc.vector.tensor_tensor(sc[:], zinv[:], RSN_sb[:, c:c + 1],
                                            AluOpType.mult)
                    nc.tensor.matmul(st2[0:1, 144:160],
                                     lhsT=sc[:],
                                     rhs=IDNF_sb[0:16, 0:16], is_transpose=True,
                                     start=True, stop=True)
                    scrow = sm_pool.tile([1, 16], F32, tag="scrow")
                    nc.vector.tensor_copy(scrow[:], st2[0:1, 144:160])
                    nc.tensor.matmul(st2[:, 128:144],
                                     lhsT=ONES_sb[:],
                                     rhs=scrow[:],
                                     start=True, stop=True)
                    screp = st2[:, 128:144].rearrange("p (s a) -> p s a", a=2) \
                        .unsqueeze(2).broadcast_to([128, CH_SEGS, TPS, 2])
                    Ew = dE_pool.tile([128, 2 * TILES_CH], BF16, tag="Ew")
                    nc.vector.tensor_tensor(
                        Ew[:].rearrange("p (s r a) -> p s r a",
                                        s=CH_SEGS, r=TPS, a=2),
                        E[:].rearrange("p (s r a) -> p s r a",
                                       s=CH_SEGS, r=TPS, a=2),
                        screp, AluOpType.mult)

                # ---- pools:  pT[feat, (h, s, a)] += x_half^T @ Ew_cols ----
                pp = ppsum.tile([128, 32], F32, tag="pp")
                for s in range(CH_SEGS if "nopool" not in ablate else 0):
                    for hh in range(2):
                        for r in range(TPS):
                            t = s * TPS + r
                            nc.tensor.matmul(
                                pp[:, hh * 16 + 2 * s: hh * 16 + 2 * s + 2],
                                lhsT=nat[:, t * H + 128 * hh:
                                         t * H + 128 * hh + 128],
                                rhs=Ew[:, 2 * t: 2 * t + 2],
                                start=(r == 0), stop=(r == TPS - 1))
                # scatter into pooled^T accumulator
                for hh in range(2):
                    grp = pp[:, hh * 16:(hh + 1) * 16].rearrange(
                        "p (s a) -> p a s", a=2)
                    for a in range(2):
                        nc.vector.tensor_copy(
                            pTsb[:, (a * 2 + hh) * SEGS + c * CH_SEGS:
                                 (a * 2 + hh) * SEGS + (c + 1) * CH_SEGS],
                            grp[:, a])

            # ---- output projection ----
            yps = tpsum.tile([128, H], F32, tag="tp")
            for f2b in range(4):
                nc.tensor.matmul(yps[0:SEGS, :],
                                 lhsT=pTsb[:, f2b * SEGS:(f2b + 1) * SEGS]
                                 ,
                                 rhs=OW_sb[:, f2b * H:(f2b + 1) * H]
                                 ,
                                 start=(f2b == 0), stop=False)
            nc.tensor.matmul(yps[0:SEGS, :],
                             lhsT=ONES_sb[:, 0:SEGS],
                             rhs=OB_sb[:],
                             start=False, stop=True)
            ysb = acc_pool.tile([SEGS, H], F32, tag="y")
            nc.scalar.copy(ysb[:], yps[0:SEGS, :])
            nc.sync.dma_start(y_d[:], ysb[:])

    nc.compile()
    return nc


def _host_prep(x, batch, lysine_mask, gw1, gb1, gw2, gb2,
               lw1, lb1, lw2, lb2, ow, ob, B=1024, n_cores=N_CORES):
    """Build per-core input maps. Returns (in_maps, L, segs_per_core, B)."""
    N = x.shape[0]
    batch = np.asarray(batch).astype(np.int64)
    segs_per_core = B // n_cores
    offs = np.searchsorted(batch, np.arange(B + 1))
    lens = np.diff(offs)
    maxlen = int(lens.max())
    L = max(128 * int(math.ceil(maxlen / 128.0)), 256)
    TPS = L // 128
    CH_SEGS = 8
    NCHUNK = segs_per_core // CH_SEGS
    TILES_CH = CH_SEGS * TPS

    x = np.asarray(x, dtype=np.float32)
    lys = np.asarray(lysine_mask).astype(bool)

    # equal-length padded x (bf16) + additive masks
    x_eq = np.zeros((B, L, H), dtype=ml_dtypes.bfloat16)
    a_eq = np.full((B, L, 2), NEG, dtype=np.float32)
    for s in range(B):
        n = int(lens[s])
        if n == 0:
            continue
        sl = slice(int(offs[s]), int(offs[s]) + n)
        x_eq[s, :n] = x[sl]
        a_eq[s, :n, 0] = float(gb2[0])
        a_eq[s, :n, 1] = np.where(lys[sl], float(lb2[0]), NEG)

    rsn = 1.0 / np.sqrt(np.maximum(lens, 1).astype(np.float32))

    # weights (shared)
    w1 = np.concatenate([gw1[:128], gw1[128:], lw1[:128], lw1[128:]],
                        axis=1).astype(ml_dtypes.bfloat16)  # [128, 512]
    b1 = np.stack([gb1, lb1], axis=1).astype(np.float32)  # [128, 2]
    gl2 = np.concatenate([gw2, lw2], axis=1).astype(ml_dtypes.bfloat16)
    # ow rows f2 = a*256 + h*128 + c  ->  block (a*2+h)
    ow_blocks = np.concatenate(
        [ow[0:128], ow[128:256], ow[256:384], ow[384:512]],
        axis=1).astype(np.float32)  # [128, 1024]
    ob_r = np.asarray(ob, dtype=np.float32).reshape(1, H)
    ones = np.ones((1, 128), dtype=np.float32)
    idn = np.eye(128, dtype=ml_dtypes.bfloat16)
    idnf = np.eye(128, dtype=np.float32)

    in_maps = []
    for core in range(n_cores):
        s0 = core * segs_per_core
        xs = x_eq[s0:s0 + segs_per_core]  # [SEGS, L, H]
        # -> [NCHUNK, 128, TILES_CH*H]: chunk c, partition p, col t*H+f
        # node within chunk: t*128+p ; t = (s_loc*L + j)//128
        xc = xs.reshape(NCHUNK, TILES_CH, 128, H).transpose(0, 2, 1, 3)
        xc = np.ascontiguousarray(xc).reshape(NCHUNK, 128, TILES_CH * H)
        asl = a_eq[s0:s0 + segs_per_core]  # [SEGS, L, 2]
        ac = asl.reshape(NCHUNK, TILES_CH, 128, 2).transpose(2, 0, 1, 3)
        ac = np.ascontiguousarray(ac).reshape(128, 2 * segs_per_core * TPS)
        rs = rsn[s0:s0 + segs_per_core].reshape(NCHUNK, CH_SEGS)
        rsc = np.repeat(rs.T[:, None, :], 2, axis=1).reshape(
            2 * CH_SEGS, NCHUNK).astype(np.float32)
        # rows must be 2*s_loc+a  (same value both a)
        rsc = np.ascontiguousarray(rsc)
        in_maps.append({
            "x": xc, "amask": ac, "rsn": rsc, "w1": w1, "b1": b1,
            "gl2": gl2, "ow": ow_blocks, "ob": ob_r, "ones": ones,
            "idn": idn, "idnf": idnf,
        })
    return in_maps, L, segs_per_core, B


def kernel(**inputs):
    x = np.asarray(inputs["x"])
    in_maps, L, segs_per_core, B = _host_prep(
        x, inputs["batch"], inputs["lysine_mask"],
        np.asarray(inputs["gw1"], np.float32), np.asarray(inputs["gb1"], np.float32),
        np.asarray(inputs["gw2"], np.float32), np.asarray(inputs["gb2"], np.float32),
        np.asarray(inputs["lw1"], np.float32), np.asarray(inputs["lb1"], np.float32),
        np.asarray(inputs["lw2"], np.float32), np.asarray(inputs["lb2"], np.float32),
        np.asarray(inputs["ow"], np.float32), np.asarray(inputs["ob"], np.float32))

    key = (L, segs_per_core)
    if key not in _cache:
        _cache[key] = _build(L, segs_per_core)
    nc = _cache[key]

    res = run_bass_kernel_spmd(nc, in_maps, core_ids=list(range(N_CORES)))
    out = np.concatenate([res.results[c]["y"] for c in range(N_CORES)], axis=0)
    return out.astype(np.float32)



# revision 2
# speedup vs baseline: 105.1810x; 105.1810x over previous
"""Trainium2 Bass kernel for segment-attention pooling (EquivariantPooling).

Math (per reference):
  g = batch_softmax(tanh(x@gw1+gb1)@gw2+gb2);  global_pool = segsum(x*g)
  l = batch_softmax(mask(tanh(x@lw1+lb1)@lw2+lb2));  lys_pool = segsum(x*l)
  out = (concat(global_pool, lys_pool)/sqrt(n_seg)) @ ow + ob

Strategy: batch ids are sorted -> contiguous segments. Host splits the 1024
segments into 8 groups of 128 (one per core), pads every segment to a common
length L (multiple of 128) and uploads:
  - nat: node-major bf16 copy of x (for the pooling matmuls)
  - xT8: feature-major fp8-e3m4 copy of x*2 (for the attention MLP; the
    1/(2*16) dequant scale is folded into the tanh activation's affine input
    scale).  e3m4 keeps 4 mantissa bits; x*2 / w1*16 center the values in its
    normal range.
The device program is identical on all cores (SPMD); only data differs.
"""

import math
import os

import numpy as np
import ml_dtypes

import concourse.bass as bass
import concourse.tile as tile
import concourse.mybir as mybir
from concourse import bacc
from concourse.alu_op_type import AluOpType
from concourse.bass_utils import run_bass_kernel_spmd

F32 = mybir.dt.float32
BF16 = mybir.dt.bfloat16
FP8 = mybir.dt.float8e3
AX = mybir.AxisListType.X
AF = mybir.ActivationFunctionType

N_CORES = 8
H = 256
NEG = -1.0e9
XSC = 2.0     # x pre-scale for e3m4
WSC = 16.0    # w1 pre-scale for e3m4

_cache = {}


def _build(L, segs_per_core):
    """Build the per-core Bass program. Returns nc."""
    TPS = L // 128            # tiles per segment
    CH_SEGS = 8               # segments per chunk
    NCHUNK = segs_per_core // CH_SEGS
    TILES_CH = CH_SEGS * TPS  # tiles per chunk (node tiles)
    NODES_CH = 128 * TILES_CH
    NGRP = NODES_CH // 512    # 512-node groups per chunk
    NHB = NGRP                # [128,1024] tanh tiles per chunk (2 attns)
    SEGS = segs_per_core

    nc = bacc.Bacc("TRN2", target_bir_lowering=False, debug=False,
                   num_devices=N_CORES)

    x_d = nc.dram_tensor("x", [NCHUNK, 128, TILES_CH * H], BF16,
                         kind="ExternalInput").ap()
    xt_d = nc.dram_tensor("xt8", [NCHUNK, 128, 2 * NODES_CH], FP8,
                          kind="ExternalInput").ap()
    a_d = nc.dram_tensor("amask", [128, 2 * SEGS * TPS], F32,
                         kind="ExternalInput").ap()
    rsn_d = nc.dram_tensor("rsn", [2 * CH_SEGS, NCHUNK], F32,
                           kind="ExternalInput").ap()
    w1_d = nc.dram_tensor("w1", [128, 512], FP8, kind="ExternalInput").ap()
    b1_d = nc.dram_tensor("b1", [128, 2], F32, kind="ExternalInput").ap()
    gl2_d = nc.dram_tensor("gl2", [128, 4], BF16, kind="ExternalInput").ap()
    ow_d = nc.dram_tensor("ow", [128, 4 * H], F32, kind="ExternalInput").ap()
    ob_d = nc.dram_tensor("ob", [1, H], F32, kind="ExternalInput").ap()
    ones_d = nc.dram_tensor("ones", [1, 128], F32, kind="ExternalInput").ap()
    idnf_d = nc.dram_tensor("idnf", [128, 128], F32, kind="ExternalInput").ap()
    y_d = nc.dram_tensor("y", [SEGS, H], F32, kind="ExternalOutput").ap()

    with tile.TileContext(nc) as tc:
        with (
            tc.tile_pool(name="const", bufs=1) as cpool,
            tc.tile_pool(name="nat", bufs=2) as nat_pool,
            tc.tile_pool(name="xT", bufs=2) as xT_pool,
            tc.tile_pool(name="h", bufs=2) as h_pool,
            tc.tile_pool(name="dE", bufs=2) as dE_pool,
            tc.tile_pool(name="small", bufs=3) as sm_pool,
            tc.tile_pool(name="acc", bufs=1) as acc_pool,
            tc.tile_pool(name="hp", bufs=2, space="PSUM") as hpsum,
            tc.tile_pool(name="sp", bufs=1, space="PSUM") as spsum,
            tc.tile_pool(name="pp", bufs=1, space="PSUM") as ppsum,
            tc.tile_pool(name="st", bufs=2, space="PSUM") as stpsum,
        ):
            # ---- constants ----
            A_sb = cpool.tile([128, 2 * SEGS * TPS], F32, tag="A")
            nc.sync.dma_start(A_sb[:], a_d[:])
            RSN_sb = cpool.tile([2 * CH_SEGS, NCHUNK], F32, tag="RSN")
            nc.sync.dma_start(RSN_sb[:], rsn_d[:])
            W1_sb = cpool.tile([128, 512], FP8, tag="W1")
            nc.sync.dma_start(W1_sb[:], w1_d[:])
            B1_sb = cpool.tile([128, 2], F32, tag="B1")
            nc.sync.dma_start(B1_sb[:], b1_d[:])
            GL2_sb = cpool.tile([128, 4], BF16, tag="GL2")
            nc.sync.dma_start(GL2_sb[:], gl2_d[:])
            OW_sb = cpool.tile([128, 4 * H], F32, tag="OW")
            nc.sync.dma_start(OW_sb[:], ow_d[:])
            OB_sb = cpool.tile([1, H], F32, tag="OB")
            nc.sync.dma_start(OB_sb[:], ob_d[:])
            ONES_sb = cpool.tile([1, 128], F32, tag="ONES")
            nc.sync.dma_start(ONES_sb[:], ones_d[:])
            IDNF_sb = cpool.tile([128, 128], F32, tag="IDNF")
            nc.sync.dma_start(IDNF_sb[:], idnf_d[:])

            # pooled^T accumulator: cols (a*2+h)*SEGS + seg
            pTsb = acc_pool.tile([128, 4 * SEGS], F32, tag="pT")

            for c in range(NCHUNK):
                # ---- load chunk ----
                nat = nat_pool.tile([128, TILES_CH * H], BF16, tag="nat")
                nc.sync.dma_start(nat[:], x_d[c])
                xT8 = xT_pool.tile([128, 2 * NODES_CH], FP8, tag="xT8")
                nc.sync.dma_start(xT8[:], xt_d[c])

                # ---- L1 + tanh ----
                # hidden stream col = a*NODES_CH + node_local
                hsb = h_pool.tile([128, 2 * NODES_CH], BF16, tag="hsb")
                for hb in range(NHB):
                    a = hb // (NHB // 2)
                    hp = hpsum.tile([128, 1024], F32, tag="hp")
                    for h2 in range(2):
                        g = (hb * 2 + h2) % NGRP
                        for k in range(2):
                            nc.tensor.matmul(
                                hp[:, h2 * 512:(h2 + 1) * 512],
                                lhsT=W1_sb[:, (a * 2 + k) * 128:
                                           (a * 2 + k + 1) * 128],
                                rhs=xT8[:, k * NODES_CH + g * 512:
                                        k * NODES_CH + (g + 1) * 512],
                                start=(k == 0), stop=(k == 1))
                    nc.scalar.activation(
                        hsb[:, hb * 1024:(hb + 1) * 1024], hp[:],
                        AF.Tanh, bias=B1_sb[:, a:a + 1],
                        scale=1.0 / (XSC * WSC))

                # ---- L2: s_ps[node, (t,a)] ----
                s_ps = spsum.tile([128, 2 * TILES_CH], F32, tag="s")
                for t in range(TILES_CH):
                    for a in range(2):
                        nc.tensor.matmul(
                            s_ps[:, 2 * t: 2 * t + 2],
                            lhsT=hsb[:, a * NODES_CH + t * 128:
                                     a * NODES_CH + (t + 1) * 128],
                            rhs=GL2_sb[:, 2 * a: 2 * a + 2],
                            start=(a == 0), stop=(a == 1))

                # ---- segment softmax stats ----
                # d = s + A  (A holds -1e9 pad/lysine masks, gb2/lb2)
                d = dE_pool.tile([128, 2 * TILES_CH], F32, tag="d")
                nc.vector.tensor_tensor(d[:], s_ps[:],
                                        A_sb[:, c * 2 * TILES_CH:
                                             (c + 1) * 2 * TILES_CH],
                                        AluOpType.add)
                d4 = d[:].rearrange("p (s r a) -> p s a r",
                                    s=CH_SEGS, r=TPS, a=2)
                M1 = sm_pool.tile([128, 16], F32, tag="M1")
                nc.vector.tensor_reduce(
                    M1[:].rearrange("p (s a) -> p s a", a=2), d4,
                    axis=AX, op=AluOpType.max)
                # cross-partition reduce via PE transpose
                st = stpsum.tile([128, 160], F32, tag="st")
                nc.tensor.matmul(st[0:16, 0:128],
                                 lhsT=M1[:],
                                 rhs=IDNF_sb[:], is_transpose=True,
                                 start=True, stop=True)
                mcol = sm_pool.tile([16, 1], F32, tag="mcol")
                nc.vector.tensor_reduce(mcol[:], st[0:16, 0:128], axis=AX,
                                        op=AluOpType.max)
                nc.vector.tensor_scalar_max(mcol[:], mcol[:], 0.0)
                nc.tensor.matmul(st[0:1, 144:160],
                                 lhsT=mcol[:],
                                 rhs=IDNF_sb[0:16, 0:16], is_transpose=True,
                                 start=True, stop=True)
                mrow = sm_pool.tile([1, 16], F32, tag="mrow")
                nc.vector.tensor_copy(mrow[:], st[0:1, 144:160])
                nc.tensor.matmul(st[:, 128:144],
                                 lhsT=ONES_sb[:],
                                 rhs=mrow[:],
                                 start=True, stop=True)
                mrep = st[:, 128:144].rearrange("p (s a) -> p s a", a=2) \
                    .unsqueeze(2).broadcast_to([128, CH_SEGS, TPS, 2])
                d2 = dE_pool.tile([128, 2 * TILES_CH], F32, tag="d2")
                d24 = d2[:].rearrange("p (s r a) -> p s r a",
                                      s=CH_SEGS, r=TPS, a=2)
                nc.vector.tensor_tensor(
                    d24, d[:].rearrange("p (s r a) -> p s r a",
                                        s=CH_SEGS, r=TPS, a=2),
                    mrep, AluOpType.subtract)
                # e = exp(d2)
                E = dE_pool.tile([128, 2 * TILES_CH], BF16, tag="E")
                nc.scalar.activation(E[:], d2[:], AF.Exp)
                Z1 = sm_pool.tile([128, 16], F32, tag="Z1")
                nc.vector.tensor_reduce(
                    Z1[:].rearrange("p (s a) -> p s a", a=2),
                    E[:].rearrange("p (s r a) -> p s a r",
                                   s=CH_SEGS, r=TPS, a=2),
                    axis=AX, op=AluOpType.add)
                st2 = stpsum.tile([128, 160], F32, tag="st")
                nc.tensor.matmul(st2[0:16, 0:128],
                                 lhsT=Z1[:],
                                 rhs=IDNF_sb[:], is_transpose=True,
                                 start=True, stop=True)
                zcol = sm_pool.tile([16, 1], F32, tag="zcol")
                nc.vector.tensor_reduce(zcol[:], st2[0:16, 0:128], axis=AX,
                                        op=AluOpType.add)
                nc.vector.tensor_scalar_add(zcol[:], zcol[:], 1.0e-8)
                zinv = sm_pool.tile([16, 1], F32, tag="zinv")
                nc.vector.reciprocal(zinv[:], zcol[:])
                sc = sm_pool.tile([16, 1], F32, tag="sc")
                nc.vector.tensor_tensor(sc[:], zinv[:], RSN_sb[:, c:c + 1],
                                        AluOpType.mult)
                nc.tensor.matmul(st2[0:1, 144:160],
                                 lhsT=sc[:],
                                 rhs=IDNF_sb[0:16, 0:16], is_transpose=True,
                                 start=True, stop=True)
                scrow = sm_pool.tile([1, 16], F32, tag="scrow")
                nc.vector.tensor_copy(scrow[:], st2[0:1, 144:160])
                nc.tensor.matmul(st2[:, 128:144],
                                 lhsT=ONES_sb[:],
                                 rhs=scrow[:],
                                 start=True, stop=True)
                screp = st2[:, 128:144].rearrange("p (s a) -> p s a", a=2) \
                    .unsqueeze(2).broadcast_to([128, CH_SEGS, TPS, 2])
                Ew = dE_pool.tile([128, 2 * TILES_CH], BF16, tag="Ew")
                nc.vector.tensor_tensor(
                    Ew[:].rearrange("p (s r a) -> p s r a",
                                    s=CH_SEGS, r=TPS, a=2),
                    E[:].rearrange("p (s r a) -> p s r a",
                                   s=CH_SEGS, r=TPS, a=2),
                    screp, AluOpType.mult)

                # ---- pools:  pT[feat, (h, s, a)] += x_half^T @ Ew_cols ----
                pp = ppsum.tile([128, 32], F32, tag="pp")
                for s in range(CH_SEGS):
                    for hh in range(2):
                        for r in range(TPS):
                            t = s * TPS + r
                            nc.tensor.matmul(
                                pp[:, hh * 16 + 2 * s: hh * 16 + 2 * s + 2],
                                lhsT=nat[:, t * H + 128 * hh:
                                         t * H + 128 * hh + 128],
                                rhs=Ew[:, 2 * t: 2 * t + 2],
                                start=(r == 0), stop=(r == TPS - 1))
                # scatter into pooled^T accumulator
                for hh in range(2):
                    grp = pp[:, hh * 16:(hh + 1) * 16].rearrange(
                        "p (s a) -> p a s", a=2)
                    for a in range(2):
                        nc.vector.tensor_copy(
                            pTsb[:, (a * 2 + hh) * SEGS + c * CH_SEGS:
                                 (a * 2 + hh) * SEGS + (c + 1) * CH_SEGS],
                            grp[:, a])

            # ---- output projection ----
            yps = hpsum.tile([128, H], F32, tag="hp")
            for f2b in range(4):
                nc.tensor.matmul(yps[0:SEGS, :],
                                 lhsT=pTsb[:, f2b * SEGS:(f2b + 1) * SEGS],
                                 rhs=OW_sb[:, f2b * H:(f2b + 1) * H],
                                 start=(f2b == 0), stop=False)
            nc.tensor.matmul(yps[0:SEGS, :],
                             lhsT=ONES_sb[:, 0:SEGS],
                             rhs=OB_sb[:],
                             start=False, stop=True)
            ysb = acc_pool.tile([SEGS, H], F32, tag="y")
            nc.scalar.copy(ysb[:], yps[0:SEGS, :])
            nc.sync.dma_start(y_d[:], ysb[:])

    nc.compile()
    return nc


def _host_prep(x, batch, lysine_mask, gw1, gb1, gw2, gb2,
               lw1, lb1, lw2, lb2, ow, ob, B=1024, n_cores=N_CORES):
    """Build per-core input maps. Returns (in_maps, L, segs_per_core, B)."""
    N = x.shape[0]
    batch = np.asarray(batch).astype(np.int64)
    segs_per_core = B // n_cores
    offs = np.searchsorted(batch, np.arange(B + 1))
    lens = np.diff(offs)
    maxlen = int(lens.max())
    L = max(128 * int(math.ceil(maxlen / 128.0)), 256)
    TPS = L // 128
    CH_SEGS = 8
    NCHUNK = segs_per_core // CH_SEGS
    TILES_CH = CH_SEGS * TPS
    NODES_CH = 128 * TILES_CH

    x = np.asarray(x, dtype=np.float32)
    lys = np.asarray(lysine_mask).astype(bool)

    # equal-length padded x (bf16) + additive masks, vectorized scatter
    pos = np.arange(N) - offs[batch]            # position within segment
    dest = batch * L + pos                      # row in [B*L]
    x_eq = np.zeros((B * L, H), dtype=ml_dtypes.bfloat16)
    x_eq[dest] = x
    x8_eq = np.zeros((B * L, H), dtype=ml_dtypes.float8_e3m4)
    x8_eq[dest] = (x * XSC).astype(ml_dtypes.float8_e3m4)
    a_eq = np.full((B * L, 2), NEG, dtype=np.float32)
    a_eq[dest, 0] = float(gb2[0])
    a_eq[dest, 1] = np.where(lys, float(lb2[0]), NEG)
    x_eq = x_eq.reshape(B, L, H)
    x8_eq = x8_eq.reshape(B, L, H)
    a_eq = a_eq.reshape(B, L, 2)

    rsn = 1.0 / np.sqrt(np.maximum(lens, 1).astype(np.float32))

    # weights (shared)
    w1 = np.concatenate([gw1[:128], gw1[128:], lw1[:128], lw1[128:]],
                        axis=1)
    w18 = (w1 * WSC).astype(ml_dtypes.float8_e3m4)  # [128, 512]
    b1 = np.stack([gb1, lb1], axis=1).astype(np.float32)  # [128, 2]
    z = np.zeros_like(gw2)
    gl2p = np.concatenate([gw2, z, z, lw2], axis=1).astype(
        ml_dtypes.bfloat16)  # [128, 4]: (a=0: [gw2|0]), (a=1: [0|lw2])
    ow_blocks = np.concatenate(
        [ow[0:128], ow[128:256], ow[256:384], ow[384:512]],
        axis=1).astype(np.float32)  # [128, 1024]
    ob_r = np.asarray(ob, dtype=np.float32).reshape(1, H)
    ones = np.ones((1, 128), dtype=np.float32)
    idnf = np.eye(128, dtype=np.float32)

    in_maps = []
    for core in range(n_cores):
        s0 = core * segs_per_core
        xs = x_eq[s0:s0 + segs_per_core]  # [SEGS, L, H]
        # nat -> [NCHUNK, 128, TILES_CH*H]: chunk c, partition p, col t*H+f
        xc = xs.reshape(NCHUNK, TILES_CH, 128, H).transpose(0, 2, 1, 3)
        xc = np.ascontiguousarray(xc).reshape(NCHUNK, 128, TILES_CH * H)
        # xT8 -> [NCHUNK, 128, 2*NODES_CH]: partition f%128, col k*NODES+n
        x8 = x8_eq[s0:s0 + segs_per_core].reshape(NCHUNK, NODES_CH, 2, 128)
        x8 = np.ascontiguousarray(x8.transpose(0, 3, 2, 1)) \
            .reshape(NCHUNK, 128, 2 * NODES_CH)
        asl = a_eq[s0:s0 + segs_per_core]  # [SEGS, L, 2]
        ac = asl.reshape(NCHUNK, TILES_CH, 128, 2).transpose(2, 0, 1, 3)
        ac = np.ascontiguousarray(ac).reshape(128, 2 * segs_per_core * TPS)
        rs = rsn[s0:s0 + segs_per_core].reshape(NCHUNK, CH_SEGS)
        rsc = np.repeat(rs.T[:, None, :], 2, axis=1).reshape(
            2 * CH_SEGS, NCHUNK).astype(np.float32)
        rsc = np.ascontiguousarray(rsc)
        in_maps.append({
            "x": xc, "xt8": x8, "amask": ac, "rsn": rsc, "w1": w18,
            "b1": b1, "gl2": gl2p, "ow": ow_blocks, "ob": ob_r,
            "ones": ones, "idnf": idnf,
        })
    return in_maps, L, segs_per_core, B


def kernel(**inputs):
    x = np.asarray(inputs["x"])
    in_maps, L, segs_per_core, B = _host_prep(
        x, inputs["batch"], inputs["lysine_mask"],
        np.asarray(inputs["gw1"], np.float32), np.asarray(inputs["gb1"], np.float32),
        np.asarray(inputs["gw2"], np.float32), np.asarray(inputs["gb2"], np.float32),
        np.asarray(inputs["lw1"], np.float32), np.asarray(inputs["lb1"], np.float32),
        np.asarray(inputs["lw2"], np.float32), np.asarray(inputs["lb2"], np.float32),
        np.asarray(inputs["ow"], np.float32), np.asarray(inputs["ob"], np.float32))

    key = (L, segs_per_core)
    if key not in _cache:
        _cache[key] = _build(L, segs_per_core)
    nc = _cache[key]

    res = run_bass_kernel_spmd(nc, in_maps, core_ids=list(range(N_CORES)))
    out = np.concatenate([res.results[c]["y"] for c in range(N_CORES)], axis=0)
    return out.astype(np.float32)


# revision 4
# speedup vs baseline: 112.9007x; 1.0734x over previous
"""Trainium2 Bass kernel for segment-attention pooling (EquivariantPooling).

Math (per reference):
  g = batch_softmax(tanh(x@gw1+gb1)@gw2+gb2);  global_pool = segsum(x*g)
  l = batch_softmax(mask(tanh(x@lw1+lb1)@lw2+lb2));  lys_pool = segsum(x*l)
  out = (concat(global_pool, lys_pool)/sqrt(n_seg)) @ ow + ob

Strategy: batch ids are sorted -> contiguous segments. Host splits the 1024
segments into 8 groups of 128 (one per core), pads every segment to a common
length L (multiple of 128) and uploads:
  - nat: node-major bf16 copy of x (for the pooling matmuls)
  - xT8: feature-major fp8-e3m4 copy of x*2 (for the attention MLP; the
    1/(2*16) dequant scale is folded into the tanh activation's affine input
    scale).  e3m4 keeps 4 mantissa bits; x*2 / w1*16 center the values in its
    normal range.
The device program is identical on all cores (SPMD); only data differs.
"""

import math
import os

import numpy as np
import ml_dtypes

import concourse.bass as bass
import concourse.tile as tile
import concourse.mybir as mybir
from concourse import bacc
from concourse.alu_op_type import AluOpType
from concourse.bass_utils import run_bass_kernel_spmd

F32 = mybir.dt.float32
BF16 = mybir.dt.bfloat16
FP8 = mybir.dt.float8e3
AX = mybir.AxisListType.X
AF = mybir.ActivationFunctionType

N_CORES = 8
H = 256
NEG = -1.0e9
XSC = 2.0     # x pre-scale for e3m4
WSC = 16.0    # w1 pre-scale for e3m4

_cache = {}


def _build(L, segs_per_core):
    """Build the per-core Bass program. Returns nc."""
    TPS = L // 128            # tiles per segment
    CH_SEGS = 8               # segments per chunk
    NCHUNK = segs_per_core // CH_SEGS
    TILES_CH = CH_SEGS * TPS  # tiles per chunk (node tiles)
    NODES_CH = 128 * TILES_CH
    NGRP = NODES_CH // 512    # 512-node groups per chunk
    NHB = NGRP                # [128,1024] tanh tiles per chunk (2 attns)
    SEGS = segs_per_core

    nc = bacc.Bacc("TRN2", target_bir_lowering=False, debug=False,
                   num_devices=N_CORES)

    x_d = nc.dram_tensor("x", [NCHUNK, 128, TILES_CH * H], BF16,
                         kind="ExternalInput").ap()
    xt_d = nc.dram_tensor("xt8", [NCHUNK, 128, 2 * NODES_CH], FP8,
                          kind="ExternalInput").ap()
    a_d = nc.dram_tensor("amask", [128, 2 * SEGS * TPS], F32,
                         kind="ExternalInput").ap()
    rsn_d = nc.dram_tensor("rsn", [2 * CH_SEGS, NCHUNK], F32,
                           kind="ExternalInput").ap()
    w1_d = nc.dram_tensor("w1", [128, 512], FP8, kind="ExternalInput").ap()
    b1_d = nc.dram_tensor("b1", [128, 2], F32, kind="ExternalInput").ap()
    gl2_d = nc.dram_tensor("gl2", [128, 4], BF16, kind="ExternalInput").ap()
    ow_d = nc.dram_tensor("ow", [128, 4 * H], F32, kind="ExternalInput").ap()
    ob_d = nc.dram_tensor("ob", [1, H], F32, kind="ExternalInput").ap()
    ones_d = nc.dram_tensor("ones", [1, 128], F32, kind="ExternalInput").ap()
    idnf_d = nc.dram_tensor("idnf", [128, 128], F32, kind="ExternalInput").ap()
    y_d = nc.dram_tensor("y", [SEGS, H], F32, kind="ExternalOutput").ap()

    with tile.TileContext(nc) as tc:
        with (
            tc.tile_pool(name="const", bufs=1) as cpool,
            tc.tile_pool(name="nat", bufs=2) as nat_pool,
            tc.tile_pool(name="xT", bufs=2) as xT_pool,
            tc.tile_pool(name="h", bufs=2) as h_pool,
            tc.tile_pool(name="dE", bufs=2) as dE_pool,
            tc.tile_pool(name="small", bufs=3) as sm_pool,
            tc.tile_pool(name="acc", bufs=1) as acc_pool,
            tc.tile_pool(name="hp", bufs=2, space="PSUM") as hpsum,
            tc.tile_pool(name="sp", bufs=2, space="PSUM") as spsum,
            tc.tile_pool(name="pp", bufs=1, space="PSUM") as ppsum,
            tc.tile_pool(name="st", bufs=1, space="PSUM") as stpsum,
        ):
            # ---- constants ----
            A_sb = cpool.tile([128, 2 * SEGS * TPS], F32, tag="A")
            nc.sync.dma_start(A_sb[:], a_d[:])
            RSN_sb = cpool.tile([2 * CH_SEGS, NCHUNK], F32, tag="RSN")
            nc.sync.dma_start(RSN_sb[:], rsn_d[:])
            W1_sb = cpool.tile([128, 512], FP8, tag="W1")
            nc.sync.dma_start(W1_sb[:], w1_d[:])
            B1_sb = cpool.tile([128, 2], F32, tag="B1")
            nc.sync.dma_start(B1_sb[:], b1_d[:])
            GL2_sb = cpool.tile([128, 4], BF16, tag="GL2")
            nc.sync.dma_start(GL2_sb[:], gl2_d[:])
            OW_sb = cpool.tile([128, 4 * H], F32, tag="OW")
            nc.sync.dma_start(OW_sb[:], ow_d[:])
            OB_sb = cpool.tile([1, H], F32, tag="OB")
            nc.sync.dma_start(OB_sb[:], ob_d[:])
            ONES_sb = cpool.tile([1, 128], F32, tag="ONES")
            nc.sync.dma_start(ONES_sb[:], ones_d[:])
            IDNF_sb = cpool.tile([128, 128], F32, tag="IDNF")
            nc.sync.dma_start(IDNF_sb[:], idnf_d[:])

            # pooled^T accumulator: cols (a*2+h)*SEGS + seg
            pTsb = acc_pool.tile([128, 4 * SEGS], F32, tag="pT")

            def front(c):
                """DMA + L1/tanh + L2 for chunk c; returns (nat, s_ps)."""
                nat = nat_pool.tile([128, TILES_CH * H], BF16, tag="nat")
                nc.sync.dma_start(nat[:], x_d[c])
                xT8 = xT_pool.tile([128, 2 * NODES_CH], FP8, tag="xT8")
                nc.sync.dma_start(xT8[:], xt_d[c])

                # L1 + tanh; hidden stream col = a*NODES_CH + node_local
                hsb = h_pool.tile([128, 2 * NODES_CH], BF16, tag="hsb")
                for hb in range(NHB):
                    a = hb // (NHB // 2)
                    hp = hpsum.tile([128, 1024], F32, tag="hp")
                    for h2 in range(2):
                        g = (hb * 2 + h2) % NGRP
                        for k in range(2):
                            nc.tensor.matmul(
                                hp[:, h2 * 512:(h2 + 1) * 512],
                                lhsT=W1_sb[:, (a * 2 + k) * 128:
                                           (a * 2 + k + 1) * 128],
                                rhs=xT8[:, k * NODES_CH + g * 512:
                                        k * NODES_CH + (g + 1) * 512],
                                start=(k == 0), stop=(k == 1))
                    nc.scalar.activation(
                        hsb[:, hb * 1024:(hb + 1) * 1024], hp[:],
                        AF.Tanh, bias=B1_sb[:, a:a + 1],
                        scale=1.0 / (XSC * WSC))

                # L2: s_ps[node, (t,a)]
                s_ps = spsum.tile([128, 2 * TILES_CH], F32, tag="s")
                for t in range(TILES_CH):
                    for a in range(2):
                        nc.tensor.matmul(
                            s_ps[:, 2 * t: 2 * t + 2],
                            lhsT=hsb[:, a * NODES_CH + t * 128:
                                     a * NODES_CH + (t + 1) * 128],
                            rhs=GL2_sb[:, 2 * a: 2 * a + 2],
                            start=(a == 0), stop=(a == 1))
                return nat, s_ps

            def back(c, nat, s_ps):
                """Softmax weights + pooling for chunk c.

                Max-free softmax: |s| <= sum|w2| ~ 10, so exp never
                overflows and the reference's max-subtraction cancels
                exactly (denominator 1e-8 shift is O(1e-8) relative).
                """
                d = dE_pool.tile([128, 2 * TILES_CH], F32, tag="d")
                nc.vector.tensor_tensor(d[:], s_ps[:],
                                        A_sb[:, c * 2 * TILES_CH:
                                             (c + 1) * 2 * TILES_CH],
                                        AluOpType.add)
                E = dE_pool.tile([128, 2 * TILES_CH], BF16, tag="E")
                nc.scalar.activation(E[:], d[:], AF.Exp)
                Z1 = sm_pool.tile([128, 16], F32, tag="Z1")
                nc.vector.tensor_reduce(
                    Z1[:].rearrange("p (s a) -> p s a", a=2),
                    E[:].rearrange("p (s r a) -> p s a r",
                                   s=CH_SEGS, r=TPS, a=2),
                    axis=AX, op=AluOpType.add)
                st = stpsum.tile([128, 160], F32, tag="st")
                nc.tensor.matmul(st[0:16, 0:128],
                                 lhsT=Z1[:],
                                 rhs=IDNF_sb[:], is_transpose=True,
                                 start=True, stop=True)
                zcol = sm_pool.tile([16, 1], F32, tag="zcol")
                nc.vector.tensor_reduce(zcol[:], st[0:16, 0:128], axis=AX,
                                        op=AluOpType.add)
                nc.vector.tensor_scalar_add(zcol[:], zcol[:], 1.0e-8)
                zinv = sm_pool.tile([16, 1], F32, tag="zinv")
                nc.vector.reciprocal(zinv[:], zcol[:])
                sc = sm_pool.tile([16, 1], F32, tag="sc")
                nc.vector.tensor_tensor(sc[:], zinv[:], RSN_sb[:, c:c + 1],
                                        AluOpType.mult)
                nc.tensor.matmul(st[0:1, 144:160],
                                 lhsT=sc[:],
                                 rhs=IDNF_sb[0:16, 0:16], is_transpose=True,
                                 start=True, stop=True)
                scrow = sm_pool.tile([1, 16], F32, tag="scrow")
                nc.vector.tensor_copy(scrow[:], st[0:1, 144:160])
                nc.tensor.matmul(st[:, 128:144],
                                 lhsT=ONES_sb[:],
                                 rhs=scrow[:],
                                 start=True, stop=True)
                screp = st[:, 128:144].rearrange("p (s a) -> p s a", a=2) \
                    .unsqueeze(2).broadcast_to([128, CH_SEGS, TPS, 2])
                Ew = dE_pool.tile([128, 2 * TILES_CH], BF16, tag="Ew")
                nc.vector.tensor_tensor(
                    Ew[:].rearrange("p (s r a) -> p s r a",
                                    s=CH_SEGS, r=TPS, a=2),
                    E[:].rearrange("p (s r a) -> p s r a",
                                   s=CH_SEGS, r=TPS, a=2),
                    screp, AluOpType.mult)

                # pools:  pT[feat, (h, s, a)] += x_half^T @ Ew_cols
                pp = ppsum.tile([128, 32], F32, tag="pp")
                for s in range(CH_SEGS):
                    for hh in range(2):
                        for r in range(TPS):
                            t = s * TPS + r
                            nc.tensor.matmul(
                                pp[:, hh * 16 + 2 * s: hh * 16 + 2 * s + 2],
                                lhsT=nat[:, t * H + 128 * hh:
                                         t * H + 128 * hh + 128],
                                rhs=Ew[:, 2 * t: 2 * t + 2],
                                start=(r == 0), stop=(r == TPS - 1))
                # scatter into pooled^T accumulator
                for hh in range(2):
                    grp = pp[:, hh * 16:(hh + 1) * 16].rearrange(
                        "p (s a) -> p a s", a=2)
                    for a in range(2):
                        nc.vector.tensor_copy(
                            pTsb[:, (a * 2 + hh) * SEGS + c * CH_SEGS:
                                 (a * 2 + hh) * SEGS + (c + 1) * CH_SEGS],
                            grp[:, a])

            # 1-chunk software skew: PE never waits on the stats chain
            prev = None
            for c in range(NCHUNK):
                cur = front(c)
                if prev is not None:
                    back(prev[0], *prev[1])
                prev = (c, cur)
            back(prev[0], *prev[1])

            # ---- output projection ----
            yps = hpsum.tile([128, H], F32, tag="hp")
            for f2b in range(4):
                nc.tensor.matmul(yps[0:SEGS, :],
                                 lhsT=pTsb[:, f2b * SEGS:(f2b + 1) * SEGS],
                                 rhs=OW_sb[:, f2b * H:(f2b + 1) * H],
                                 start=(f2b == 0), stop=False)
            nc.tensor.matmul(yps[0:SEGS, :],
                             lhsT=ONES_sb[:, 0:SEGS],
                             rhs=OB_sb[:],
                             start=False, stop=True)
            ysb = acc_pool.tile([SEGS, H], F32, tag="y")
            nc.scalar.copy(ysb[:], yps[0:SEGS, :])
            nc.sync.dma_start(y_d[:], ysb[:])

    nc.compile()
    return nc


def _host_prep(x, batch, lysine_mask, gw1, gb1, gw2, gb2,
               lw1, lb1, lw2, lb2, ow, ob, B=1024, n_cores=N_CORES):
    """Build per-core input maps. Returns (in_maps, L, segs_per_core, B)."""
    N = x.shape[0]
    batch = np.asarray(batch).astype(np.int64)
    segs_per_core = B // n_cores
    offs = np.searchsorted(batch, np.arange(B + 1))
    lens = np.diff(offs)
    maxlen = int(lens.max())
    L = max(128 * int(math.ceil(maxlen / 128.0)), 256)
    TPS = L // 128
    CH_SEGS = 8
    NCHUNK = segs_per_core // CH_SEGS
    TILES_CH = CH_SEGS * TPS
    NODES_CH = 128 * TILES_CH

    x = np.asarray(x, dtype=np.float32)
    lys = np.asarray(lysine_mask).astype(bool)

    # equal-length padded x (bf16) + additive masks, vectorized scatter
    pos = np.arange(N) - offs[batch]            # position within segment
    dest = batch * L + pos                      # row in [B*L]
    x_eq = np.zeros((B * L, H), dtype=ml_dtypes.bfloat16)
    x_eq[dest] = x
    x8_eq = np.zeros((B * L, H), dtype=ml_dtypes.float8_e3m4)
    x8_eq[dest] = (x * XSC).astype(ml_dtypes.float8_e3m4)
    a_eq = np.full((B * L, 2), NEG, dtype=np.float32)
    a_eq[dest, 0] = float(gb2[0])
    a_eq[dest, 1] = np.where(lys, float(lb2[0]), NEG)
    x_eq = x_eq.reshape(B, L, H)
    x8_eq = x8_eq.reshape(B, L, H)
    a_eq = a_eq.reshape(B, L, 2)

    rsn = 1.0 / np.sqrt(np.maximum(lens, 1).astype(np.float32))

    # weights (shared)
    w1 = np.concatenate([gw1[:128], gw1[128:], lw1[:128], lw1[128:]],
                        axis=1)
    w18 = (w1 * WSC).astype(ml_dtypes.float8_e3m4)  # [128, 512]
    b1 = np.stack([gb1, lb1], axis=1).astype(np.float32)  # [128, 2]
    z = np.zeros_like(gw2)
    gl2p = np.concatenate([gw2, z, z, lw2], axis=1).astype(
        ml_dtypes.bfloat16)  # [128, 4]: (a=0: [gw2|0]), (a=1: [0|lw2])
    ow_blocks = np.concatenate(
        [ow[0:128], ow[128:256], ow[256:384], ow[384:512]],
        axis=1).astype(np.float32)  # [128, 1024]
    ob_r = np.asarray(ob, dtype=np.float32).reshape(1, H)
    ones = np.ones((1, 128), dtype=np.float32)
    idnf = np.eye(128, dtype=np.float32)

    in_maps = []
    for core in range(n_cores):
        s0 = core * segs_per_core
        xs = x_eq[s0:s0 + segs_per_core]  # [SEGS, L, H]
        # nat -> [NCHUNK, 128, TILES_CH*H]: chunk c, partition p, col t*H+f
        xc = xs.reshape(NCHUNK, TILES_CH, 128, H).transpose(0, 2, 1, 3)
        xc = np.ascontiguousarray(xc).reshape(NCHUNK, 128, TILES_CH * H)
        # xT8 -> [NCHUNK, 128, 2*NODES_CH]: partition f%128, col k*NODES+n
        x8 = x8_eq[s0:s0 + segs_per_core].reshape(NCHUNK, NODES_CH, 2, 128)
        x8 = np.ascontiguousarray(x8.transpose(0, 3, 2, 1)) \
            .reshape(NCHUNK, 128, 2 * NODES_CH)
        asl = a_eq[s0:s0 + segs_per_core]  # [SEGS, L, 2]
        ac = asl.reshape(NCHUNK, TILES_CH, 128, 2).transpose(2, 0, 1, 3)
        ac = np.ascontiguousarray(ac).reshape(128, 2 * segs_per_core * TPS)
        rs = rsn[s0:s0 + segs_per_core].reshape(NCHUNK, CH_SEGS)
        rsc = np.repeat(rs.T[:, None, :], 2, axis=1).reshape(
            2 * CH_SEGS, NCHUNK).astype(np.float32)
        rsc = np.ascontiguousarray(rsc)
        in_maps.append({
            "x": xc, "xt8": x8, "amask": ac, "rsn": rsc, "w1": w18,
            "b1": b1, "gl2": gl2p, "ow": ow_blocks, "ob": ob_r,
            "ones": ones, "idnf": idnf,
        })
    return in_maps, L, segs_per_core, B


def kernel(**inputs):
    x = np.asarray(inputs["x"])
    in_maps, L, segs_per_core, B = _host_prep(
        x, inputs["batch"], inputs["lysine_mask"],
        np.asarray(inputs["gw1"], np.float32), np.asarray(inputs["gb1"], np.float32),
        np.asarray(inputs["gw2"], np.float32), np.asarray(inputs["gb2"], np.float32),
        np.asarray(inputs["lw1"], np.float32), np.asarray(inputs["lb1"], np.float32),
        np.asarray(inputs["lw2"], np.float32), np.asarray(inputs["lb2"], np.float32),
        np.asarray(inputs["ow"], np.float32), np.asarray(inputs["ob"], np.float32))

    key = (L, segs_per_core)
    if key not in _cache:
        _cache[key] = _build(L, segs_per_core)
    nc = _cache[key]

    res = run_bass_kernel_spmd(nc, in_maps, core_ids=list(range(N_CORES)))
    out = np.concatenate([res.results[c]["y"] for c in range(N_CORES)], axis=0)
    return out.astype(np.float32)


# revision 6
# speedup vs baseline: 134.2508x; 1.1891x over previous
"""Trainium2 Bass kernel for segment-attention pooling (EquivariantPooling).

Math (per reference):
  g = batch_softmax(tanh(x@gw1+gb1)@gw2+gb2);  global_pool = segsum(x*g)
  l = batch_softmax(mask(tanh(x@lw1+lb1)@lw2+lb2));  lys_pool = segsum(x*l)
  out = (concat(global_pool, lys_pool)/sqrt(n_seg)) @ ow + ob

Strategy: batch ids are sorted -> contiguous segments.  The 1024 segments are
rank-sorted by length and dealt round-robin to the 8 cores, so every core gets
the same multiset of lengths; chunks of 8 same-rank-window segments share one
padded tile count (TPS), cutting pad overhead from ~31% to ~8% while keeping
the program SPMD-identical across cores.  Host uploads:
  - nat: node-major bf16 copy of x (pooling matmuls)
  - xT8: feature-major fp8-e3m4 copy of x*2 (attention MLP; the 1/(2*16)
    dequant is folded into the tanh activation's affine input scale)
Softmax is computed max-free (scores are bounded by sum|w2| ~ 10, so exp
cannot overflow and the reference's max-subtraction cancels exactly).
The host un-permutes the per-core outputs back to global segment order.
"""

import math

import numpy as np
import ml_dtypes

import concourse.bass as bass
import concourse.tile as tile
import concourse.mybir as mybir
from concourse import bacc
from concourse.alu_op_type import AluOpType
from concourse.bass_utils import run_bass_kernel_spmd

F32 = mybir.dt.float32
BF16 = mybir.dt.bfloat16
FP8 = mybir.dt.float8e3
AX = mybir.AxisListType.X
AF = mybir.ActivationFunctionType

N_CORES = 8
H = 256
NEG = -1.0e9
XSC = 2.0     # x pre-scale for e3m4
WSC = 16.0    # w1 pre-scale for e3m4
CH_SEGS = 8   # segments per chunk

_cache = {}


def _build(tps_list, segs_per_core):
    """Build the per-core Bass program for the given per-chunk tile counts."""
    tps_list = list(tps_list)
    NCHUNK = len(tps_list)
    SEGS = segs_per_core
    assert SEGS == NCHUNK * CH_SEGS
    TILES = [CH_SEGS * t for t in tps_list]          # node tiles per chunk
    TOFF = [0]
    for t in TILES:
        TOFF.append(TOFF[-1] + t)                    # tile offsets
    TT = TOFF[-1]
    NODES = [128 * t for t in TILES]
    NOFF = [128 * o for o in TOFF]
    CN = 128 * TT                                     # nodes per core

    nc = bacc.Bacc("TRN2", target_bir_lowering=False, debug=False,
                   num_devices=N_CORES)

    x_d = nc.dram_tensor("x", [128, TT * H], BF16, kind="ExternalInput").ap()
    xt_d = nc.dram_tensor("xt8", [128, 2 * CN], FP8,
                          kind="ExternalInput").ap()
    a_d = nc.dram_tensor("amask", [128, 2 * TT], F32,
                         kind="ExternalInput").ap()
    rsn_d = nc.dram_tensor("rsn", [2 * CH_SEGS, NCHUNK], F32,
                           kind="ExternalInput").ap()
    w1_d = nc.dram_tensor("w1", [128, 512], FP8, kind="ExternalInput").ap()
    b1_d = nc.dram_tensor("b1", [128, 2], F32, kind="ExternalInput").ap()
    gl2_d = nc.dram_tensor("gl2", [128, 4], BF16, kind="ExternalInput").ap()
    ow_d = nc.dram_tensor("ow", [128, 4 * H], F32, kind="ExternalInput").ap()
    ob_d = nc.dram_tensor("ob", [1, H], F32, kind="ExternalInput").ap()
    ones_d = nc.dram_tensor("ones", [1, 128], F32, kind="ExternalInput").ap()
    idnf_d = nc.dram_tensor("idnf", [128, 128], F32, kind="ExternalInput").ap()
    y_d = nc.dram_tensor("y", [SEGS, H], F32, kind="ExternalOutput").ap()

    with tile.TileContext(nc) as tc:
        with (
            tc.tile_pool(name="const", bufs=1) as cpool,
            tc.tile_pool(name="nat", bufs=2) as nat_pool,
            tc.tile_pool(name="xT", bufs=2) as xT_pool,
            tc.tile_pool(name="h", bufs=2) as h_pool,
            tc.tile_pool(name="dE", bufs=2) as dE_pool,
            tc.tile_pool(name="small", bufs=3) as sm_pool,
            tc.tile_pool(name="acc", bufs=1) as acc_pool,
            tc.tile_pool(name="hp", bufs=2, space="PSUM") as hpsum,
            tc.tile_pool(name="sp", bufs=2, space="PSUM") as spsum,
            tc.tile_pool(name="pp", bufs=1, space="PSUM") as ppsum,
            tc.tile_pool(name="st", bufs=1, space="PSUM") as stpsum,
        ):
            # ---- constants ----
            A_sb = cpool.tile([128, 2 * TT], F32, tag="A")
            nc.sync.dma_start(A_sb[:], a_d[:])
            RSN_sb = cpool.tile([2 * CH_SEGS, NCHUNK], F32, tag="RSN")
            nc.sync.dma_start(RSN_sb[:], rsn_d[:])
            W1_sb = cpool.tile([128, 512], FP8, tag="W1")
            nc.sync.dma_start(W1_sb[:], w1_d[:])
            B1_sb = cpool.tile([128, 2], F32, tag="B1")
            nc.sync.dma_start(B1_sb[:], b1_d[:])
            GL2_sb = cpool.tile([128, 4], BF16, tag="GL2")
            nc.sync.dma_start(GL2_sb[:], gl2_d[:])
            OW_sb = cpool.tile([128, 4 * H], F32, tag="OW")
            nc.sync.dma_start(OW_sb[:], ow_d[:])
            OB_sb = cpool.tile([1, H], F32, tag="OB")
            nc.sync.dma_start(OB_sb[:], ob_d[:])
            ONES_sb = cpool.tile([1, 128], F32, tag="ONES")
            nc.sync.dma_start(ONES_sb[:], ones_d[:])
            IDNF_sb = cpool.tile([128, 128], F32, tag="IDNF")
            nc.sync.dma_start(IDNF_sb[:], idnf_d[:])

            # pooled^T accumulator: cols (a*2+h)*SEGS + seg
            pTsb = acc_pool.tile([128, 4 * SEGS], F32, tag="pT")

            NAT_MAX = max(TILES) * H
            XT_MAX = 2 * max(NODES)

            def front(c):
                """DMA + L1/tanh + L2 for chunk c; returns (nat, s_ps)."""
                TPS = tps_list[c]
                TILES_C = TILES[c]
                NODES_C = NODES[c]
                NGRP = NODES_C // 512
                NHB = 2 * TPS  # [128,1024] tanh tiles ( = 2*NODES_C/1024 )

                nat = nat_pool.tile([128, NAT_MAX], BF16, tag="nat")
                nc.sync.dma_start(nat[:, 0:TILES_C * H],
                                  x_d[:, TOFF[c] * H:(TOFF[c] + TILES_C) * H])
                xT8 = xT_pool.tile([128, XT_MAX], FP8, tag="xT8")
                nc.sync.dma_start(xT8[:, 0:2 * NODES_C],
                                  xt_d[:, 2 * NOFF[c]:2 * NOFF[c] + 2 * NODES_C])

                # L1 + tanh; hidden stream col = a*NODES_C + node_local
                hsb = h_pool.tile([128, XT_MAX], BF16, tag="hsb")
                for hb in range(NHB):
                    a = hb // TPS
                    hp = hpsum.tile([128, 1024], F32, tag="hp")
                    for h2 in range(2):
                        g = (hb * 2 + h2) % NGRP
                        for k in range(2):
                            nc.tensor.matmul(
                                hp[:, h2 * 512:(h2 + 1) * 512],
                                lhsT=W1_sb[:, (a * 2 + k) * 128:
                                           (a * 2 + k + 1) * 128],
                                rhs=xT8[:, k * NODES_C + g * 512:
                                        k * NODES_C + (g + 1) * 512],
                                start=(k == 0), stop=(k == 1))
                    nc.scalar.activation(
                        hsb[:, hb * 1024:(hb + 1) * 1024], hp[:],
                        AF.Tanh, bias=B1_sb[:, a:a + 1],
                        scale=1.0 / (XSC * WSC))

                # L2: s_ps[node, (t,a)]
                s_ps = spsum.tile([128, 2 * max(TILES)], F32, tag="s")
                for t in range(TILES_C):
                    for a in range(2):
                        nc.tensor.matmul(
                            s_ps[:, 2 * t: 2 * t + 2],
                            lhsT=hsb[:, a * NODES_C + t * 128:
                                     a * NODES_C + (t + 1) * 128],
                            rhs=GL2_sb[:, 2 * a: 2 * a + 2],
                            start=(a == 0), stop=(a == 1))
                return nat, s_ps

            def back(c, nat, s_ps):
                """Softmax weights + pooling for chunk c (max-free softmax)."""
                TPS = tps_list[c]
                TILES_C = TILES[c]
                d = dE_pool.tile([128, 2 * max(TILES)], F32, tag="d")
                nc.vector.tensor_tensor(d[:, 0:2 * TILES_C],
                                        s_ps[:, 0:2 * TILES_C],
                                        A_sb[:, 2 * TOFF[c]:
                                             2 * TOFF[c] + 2 * TILES_C],
                                        AluOpType.add)
                E = dE_pool.tile([128, 2 * max(TILES)], BF16, tag="E")
                nc.scalar.activation(E[:, 0:2 * TILES_C], d[:, 0:2 * TILES_C],
                                     AF.Exp)
                Z1 = sm_pool.tile([128, 16], F32, tag="Z1")
                nc.vector.tensor_reduce(
                    Z1[:].rearrange("p (s a) -> p s a", a=2),
                    E[:, 0:2 * TILES_C].rearrange(
                        "p (s r a) -> p s a r", s=CH_SEGS, r=TPS, a=2),
                    axis=AX, op=AluOpType.add)
                st = stpsum.tile([128, 160], F32, tag="st")
                nc.tensor.matmul(st[0:16, 0:128],
                                 lhsT=Z1[:],
                                 rhs=IDNF_sb[:], is_transpose=True,
                                 start=True, stop=True)
                zcol = sm_pool.tile([16, 1], F32, tag="zcol")
                nc.vector.tensor_reduce(zcol[:], st[0:16, 0:128], axis=AX,
                                        op=AluOpType.add)
                nc.vector.tensor_scalar_add(zcol[:], zcol[:], 1.0e-8)
                zinv = sm_pool.tile([16, 1], F32, tag="zinv")
                nc.vector.reciprocal(zinv[:], zcol[:])
                sc = sm_pool.tile([16, 1], F32, tag="sc")
                nc.vector.tensor_tensor(sc[:], zinv[:], RSN_sb[:, c:c + 1],
                                        AluOpType.mult)
                nc.tensor.matmul(st[0:1, 144:160],
                                 lhsT=sc[:],
                                 rhs=IDNF_sb[0:16, 0:16], is_transpose=True,
                                 start=True, stop=True)
                scrow = sm_pool.tile([1, 16], F32, tag="scrow")
                nc.vector.tensor_copy(scrow[:], st[0:1, 144:160])
                nc.tensor.matmul(st[:, 128:144],
                                 lhsT=ONES_sb[:],
                                 rhs=scrow[:],
                                 start=True, stop=True)
                screp = st[:, 128:144].rearrange("p (s a) -> p s a", a=2) \
                    .unsqueeze(2).broadcast_to([128, CH_SEGS, TPS, 2])
                Ew = dE_pool.tile([128, 2 * max(TILES)], BF16, tag="Ew")
                nc.vector.tensor_tensor(
                    Ew[:, 0:2 * TILES_C].rearrange(
                        "p (s r a) -> p s r a", s=CH_SEGS, r=TPS, a=2),
                    E[:, 0:2 * TILES_C].rearrange(
                        "p (s r a) -> p s r a", s=CH_SEGS, r=TPS, a=2),
                    screp, AluOpType.mult)

                # pools:  pT[feat, (h, s, a)] += x_half^T @ Ew_cols
                pp = ppsum.tile([128, 32], F32, tag="pp")
                for s in range(CH_SEGS):
                    for hh in range(2):
                        for r in range(TPS):
                            t = s * TPS + r
                            nc.tensor.matmul(
                                pp[:, hh * 16 + 2 * s: hh * 16 + 2 * s + 2],
                                lhsT=nat[:, t * H + 128 * hh:
                                         t * H + 128 * hh + 128],
                                rhs=Ew[:, 2 * t: 2 * t + 2],
                                start=(r == 0), stop=(r == TPS - 1))
                # scatter into pooled^T accumulator
                for hh in range(2):
                    grp = pp[:, hh * 16:(hh + 1) * 16].rearrange(
                        "p (s a) -> p a s", a=2)
                    for a in range(2):
                        nc.vector.tensor_copy(
                            pTsb[:, (a * 2 + hh) * SEGS + c * CH_SEGS:
                                 (a * 2 + hh) * SEGS + (c + 1) * CH_SEGS],
                            grp[:, a])

            # 1-chunk software skew: PE never waits on the stats chain
            prev = None
            for c in range(NCHUNK):
                cur = front(c)
                if prev is not None:
                    back(prev[0], *prev[1])
                prev = (c, cur)
            back(prev[0], *prev[1])

            # ---- output projection ----
            yps = hpsum.tile([128, H], F32, tag="hp")
            for f2b in range(4):
                nc.tensor.matmul(yps[0:SEGS, :],
                                 lhsT=pTsb[:, f2b * SEGS:(f2b + 1) * SEGS],
                                 rhs=OW_sb[:, f2b * H:(f2b + 1) * H],
                                 start=(f2b == 0), stop=False)
            nc.tensor.matmul(yps[0:SEGS, :],
                             lhsT=ONES_sb[:, 0:SEGS],
                             rhs=OB_sb[:],
                             start=False, stop=True)
            ysb = acc_pool.tile([SEGS, H], F32, tag="y")
            nc.scalar.copy(ysb[:], yps[0:SEGS, :])
            nc.sync.dma_start(y_d[:], ysb[:])

    nc.compile()
    return nc


def _plan(batch, B):
    """Segment geometry: rank-sorted round-robin core assignment."""
    offs = np.searchsorted(batch, np.arange(B + 1))
    lens = np.diff(offs)
    order = np.argsort(lens, kind="stable")          # rank -> segment
    segs_per_core = B // N_CORES
    nchunk = segs_per_core // CH_SEGS
    sl = lens[order]
    tps_list = []
    for j in range(nchunk):
        mx = int(sl[j * CH_SEGS * N_CORES:(j + 1) * CH_SEGS * N_CORES].max())
        tps_list.append(max(int(math.ceil(mx / 128.0)), 1))
    return offs, lens, order, tuple(tps_list), segs_per_core


def _host_prep(x, batch, lysine_mask, gw1, gb1, gw2, gb2,
               lw1, lb1, lw2, lb2, ow, ob, B=1024, n_cores=N_CORES):
    """Build per-core input maps. Returns (in_maps, key, segs_per_core, B)."""
    N = x.shape[0]
    batch = np.asarray(batch).astype(np.int64)
    offs, lens, order, tps_list, segs_per_core = _plan(batch, B)
    NCHUNK = len(tps_list)
    TILES = [CH_SEGS * t for t in tps_list]
    TOFF = np.concatenate([[0], np.cumsum(TILES)]).astype(np.int64)
    TT = int(TOFF[-1])
    NODES = [128 * t for t in TILES]
    NOFF = (128 * TOFF).astype(np.int64)
    CN = 128 * TT

    x = np.asarray(x, dtype=np.float32)
    lys = np.asarray(lysine_mask).astype(bool)

    # per-segment destination bases in the global padded [8*CN] node space
    rank_of = np.empty(B, dtype=np.int64)
    rank_of[order] = np.arange(B)
    core_of = rank_of % n_cores
    slot_of = rank_of // n_cores
    chunk_of = slot_of // CH_SEGS
    sloc_of = slot_of % CH_SEGS
    tpss = np.asarray(tps_list, dtype=np.int64)
    seg_base = (core_of * CN + NOFF[chunk_of]
                + sloc_of * 128 * tpss[chunk_of])
    pos = np.arange(N) - offs[batch]
    dest = seg_base[batch] + pos

    x_all = np.zeros((n_cores * CN, H), dtype=ml_dtypes.bfloat16)
    x_all[dest] = x
    x8_all = np.zeros((n_cores * CN, H), dtype=ml_dtypes.float8_e3m4)
    x8_all[dest] = (x * XSC).astype(ml_dtypes.float8_e3m4)
    a_all = np.full((n_cores * CN, 2), NEG, dtype=np.float32)
    a_all[dest, 0] = float(gb2[0])
    a_all[dest, 1] = np.where(lys, float(lb2[0]), NEG)

    # per-(core,slot) 1/sqrt(len)
    rsn_cs = np.empty((n_cores, segs_per_core), dtype=np.float32)
    rsn_cs[core_of, slot_of] = 1.0 / np.sqrt(
        np.maximum(lens, 1).astype(np.float32))

    # weights (shared)
    w1 = np.concatenate([gw1[:128], gw1[128:], lw1[:128], lw1[128:]], axis=1)
    w18 = (w1 * WSC).astype(ml_dtypes.float8_e3m4)  # [128, 512]
    b1 = np.stack([gb1, lb1], axis=1).astype(np.float32)  # [128, 2]
    z = np.zeros_like(gw2)
    gl2p = np.concatenate([gw2, z, z, lw2], axis=1).astype(
        ml_dtypes.bfloat16)  # [128, 4]
    ow_blocks = np.concatenate(
        [ow[0:128], ow[128:256], ow[256:384], ow[384:512]],
        axis=1).astype(np.float32)  # [128, 1024]
    ob_r = np.asarray(ob, dtype=np.float32).reshape(1, H)
    ones = np.ones((1, 128), dtype=np.float32)
    idnf = np.eye(128, dtype=np.float32)

    in_maps = []
    for core in range(n_cores):
        xc_flat = np.empty((128, TT * H), dtype=ml_dtypes.bfloat16)
        x8_flat = np.empty((128, 2 * CN), dtype=ml_dtypes.float8_e3m4)
        a_flat = np.empty((128, 2 * TT), dtype=np.float32)
        for c in range(NCHUNK):
            blk = x_all[core * CN + NOFF[c]: core * CN + NOFF[c] + NODES[c]]
            xc = blk.reshape(TILES[c], 128, H).transpose(1, 0, 2)
            xc_flat[:, TOFF[c] * H:(TOFF[c] + TILES[c]) * H] = \
                xc.reshape(128, TILES[c] * H)
            b8 = x8_all[core * CN + NOFF[c]: core * CN + NOFF[c] + NODES[c]]
            x8 = b8.reshape(NODES[c], 2, 128).transpose(2, 1, 0)
            x8_flat[:, 2 * NOFF[c]:2 * NOFF[c] + 2 * NODES[c]] = \
                x8.reshape(128, 2 * NODES[c])
            ab = a_all[core * CN + NOFF[c]: core * CN + NOFF[c] + NODES[c]]
            ac = ab.reshape(TILES[c], 128, 2).transpose(1, 0, 2)
            a_flat[:, 2 * TOFF[c]:2 * TOFF[c] + 2 * TILES[c]] = \
                ac.reshape(128, 2 * TILES[c])
        rs = rsn_cs[core].reshape(NCHUNK, CH_SEGS)
        rsc = np.ascontiguousarray(
            np.repeat(rs.T[:, None, :], 2, axis=1).reshape(
                2 * CH_SEGS, NCHUNK)).astype(np.float32)
        in_maps.append({
            "x": xc_flat, "xt8": x8_flat, "amask": a_flat, "rsn": rsc,
            "w1": w18, "b1": b1, "gl2": gl2p, "ow": ow_blocks, "ob": ob_r,
            "ones": ones, "idnf": idnf,
        })
    return in_maps, tps_list, segs_per_core, B


def kernel(**inputs):
    x = np.asarray(inputs["x"])
    batch = np.asarray(inputs["batch"]).astype(np.int64)
    B = 1024
    in_maps, tps_list, segs_per_core, B = _host_prep(
        x, batch, inputs["lysine_mask"],
        np.asarray(inputs["gw1"], np.float32), np.asarray(inputs["gb1"], np.float32),
        np.asarray(inputs["gw2"], np.float32), np.asarray(inputs["gb2"], np.float32),
        np.asarray(inputs["lw1"], np.float32), np.asarray(inputs["lb1"], np.float32),
        np.asarray(inputs["lw2"], np.float32), np.asarray(inputs["lb2"], np.float32),
        np.asarray(inputs["ow"], np.float32), np.asarray(inputs["ob"], np.float32))

    key = (tps_list, segs_per_core)
    if key not in _cache:
        _cache[key] = _build(*key)
    nc = _cache[key]

    res = run_bass_kernel_spmd(nc, in_maps, core_ids=list(range(N_CORES)))

    # un-permute: core c slot i holds segment order[i*8 + c]
    _, _, order, _, _ = _plan(batch, B)
    out = np.empty((B, H), dtype=np.float32)
    for c in range(N_CORES):
        segs = order[np.arange(segs_per_core) * N_CORES + c]
        out[segs] = res.results[c]["y"]
    return out


# revision 11
# speedup vs baseline: 137.9751x; 1.0277x over previous
"""Trainium2 Bass kernel for segment-attention pooling (EquivariantPooling).

Math (per reference):
  g = batch_softmax(tanh(x@gw1+gb1)@gw2+gb2);  global_pool = segsum(x*g)
  l = batch_softmax(mask(tanh(x@lw1+lb1)@lw2+lb2));  lys_pool = segsum(x*l)
  out = (concat(global_pool, lys_pool)/sqrt(n_seg)) @ ow + ob

Strategy: batch ids are sorted -> contiguous segments.  The 1024 segments are
rank-sorted by length and dealt round-robin to the 8 cores, so every core gets
the same multiset of lengths; chunks of 8 same-rank-window segments share one
padded tile count (TPS), cutting pad overhead from ~31% to ~8% while keeping
the program SPMD-identical across cores.  Host uploads:
  - nat: node-major bf16 copy of x (pooling matmuls)
  - xT8: feature-major fp8-e3m4 copy of x*2 (attention MLP; the 1/(2*16)
    dequant is folded into the tanh activation's affine input scale)
Softmax is computed max-free (scores are bounded by sum|w2| ~ 10, so exp
cannot overflow and the reference's max-subtraction cancels exactly).
The host un-permutes the per-core outputs back to global segment order.
"""

import math

import numpy as np
import ml_dtypes

import concourse.bass as bass
import concourse.tile as tile
import concourse.mybir as mybir
from concourse import bacc
from concourse.alu_op_type import AluOpType
from concourse.bass_utils import run_bass_kernel_spmd

F32 = mybir.dt.float32
BF16 = mybir.dt.bfloat16
FP8 = mybir.dt.float8e3
AX = mybir.AxisListType.X
AF = mybir.ActivationFunctionType

N_CORES = 8
H = 256
NEG = -1.0e9
XSC = 2.0     # x pre-scale for e3m4
WSC = 16.0    # w1 pre-scale for e3m4
CH_SEGS = 8   # segments per chunk

_cache = {}


def _build(tps_list, segs_per_core):
    """Build the per-core Bass program for the given per-chunk tile counts."""
    tps_list = list(tps_list)
    NCHUNK = len(tps_list)
    SEGS = segs_per_core
    assert SEGS == NCHUNK * CH_SEGS
    TILES = [CH_SEGS * t for t in tps_list]          # node tiles per chunk
    TOFF = [0]
    for t in TILES:
        TOFF.append(TOFF[-1] + t)                    # tile offsets
    TT = TOFF[-1]
    NODES = [128 * t for t in TILES]
    NOFF = [128 * o for o in TOFF]
    CN = 128 * TT                                     # nodes per core

    nc = bacc.Bacc("TRN2", target_bir_lowering=False, debug=False,
                   num_devices=N_CORES)

    x_d = nc.dram_tensor("x", [128, TT * H], BF16, kind="ExternalInput").ap()
    xt_d = nc.dram_tensor("xt8", [128, 2 * CN], FP8,
                          kind="ExternalInput").ap()
    a_d = nc.dram_tensor("amask", [128, 2 * TT], F32,
                         kind="ExternalInput").ap()
    rsn_d = nc.dram_tensor("rsn", [2 * CH_SEGS, NCHUNK], F32,
                           kind="ExternalInput").ap()
    w1_d = nc.dram_tensor("w1", [128, 512], FP8, kind="ExternalInput").ap()
    b1_d = nc.dram_tensor("b1", [128, 2], F32, kind="ExternalInput").ap()
    gl2_d = nc.dram_tensor("gl2", [128, 4], BF16, kind="ExternalInput").ap()
    ow_d = nc.dram_tensor("ow", [128, 4 * H], F32, kind="ExternalInput").ap()
    ob_d = nc.dram_tensor("ob", [1, H], F32, kind="ExternalInput").ap()
    ones_d = nc.dram_tensor("ones", [1, 128], F32, kind="ExternalInput").ap()
    idnf_d = nc.dram_tensor("idnf", [128, 128], F32, kind="ExternalInput").ap()
    y_d = nc.dram_tensor("y", [SEGS, H], F32, kind="ExternalOutput").ap()

    with tile.TileContext(nc) as tc:
        with (
            tc.tile_pool(name="const", bufs=1) as cpool,
            tc.tile_pool(name="nat", bufs=2) as nat_pool,
            tc.tile_pool(name="xT", bufs=2) as xT_pool,
            tc.tile_pool(name="h", bufs=2) as h_pool,
            tc.tile_pool(name="dE", bufs=2) as dE_pool,
            tc.tile_pool(name="small", bufs=3) as sm_pool,
            tc.tile_pool(name="acc", bufs=1) as acc_pool,
            tc.tile_pool(name="hp", bufs=2, space="PSUM") as hpsum,
            tc.tile_pool(name="sp", bufs=2, space="PSUM") as spsum,
            tc.tile_pool(name="pp", bufs=1, space="PSUM") as ppsum,
            tc.tile_pool(name="st", bufs=1, space="PSUM") as stpsum,
        ):
            # ---- constants (L1-critical first; OW/OB loaded late) ----
            W1_sb = cpool.tile([128, 512], FP8, tag="W1")
            nc.sync.dma_start(W1_sb[:], w1_d[:])
            B1_sb = cpool.tile([128, 2], F32, tag="B1")
            nc.sync.dma_start(B1_sb[:], b1_d[:])
            GL2_sb = cpool.tile([128, 4], BF16, tag="GL2")
            nc.sync.dma_start(GL2_sb[:], gl2_d[:])
            A_sb = cpool.tile([128, 2 * TT], F32, tag="A")
            nc.sync.dma_start(A_sb[:], a_d[:])
            RSN_sb = cpool.tile([2 * CH_SEGS, NCHUNK], F32, tag="RSN")
            nc.sync.dma_start(RSN_sb[:], rsn_d[:])
            ONES_sb = cpool.tile([1, 128], F32, tag="ONES")
            nc.sync.dma_start(ONES_sb[:], ones_d[:])
            IDNF_sb = cpool.tile([128, 128], F32, tag="IDNF")
            nc.sync.dma_start(IDNF_sb[:], idnf_d[:])
            OW_sb = cpool.tile([128, 4 * H], F32, tag="OW")
            OB_sb = cpool.tile([1, H], F32, tag="OB")

            # pooled^T accumulator: cols (a*2+h)*SEGS + seg
            pTsb = acc_pool.tile([128, 4 * SEGS], F32, tag="pT")

            NAT_MAX = max(TILES) * H
            XT_MAX = 2 * max(NODES)

            def stats_stage(stage, c, s_ps, ctx):
                """One slice of the softmax-weight chain for chunk c.

                Max-free softmax: |s| <= sum|w2| ~ 10, so exp never overflows
                and the reference's max-subtraction cancels exactly."""
                TPS = tps_list[c]
                TILES_C = TILES[c]
                if stage == 0:
                    d = dE_pool.tile([128, 2 * max(TILES)], F32, tag="d")
                    nc.vector.tensor_tensor(
                        d[:, 0:2 * TILES_C], s_ps[:, 0:2 * TILES_C],
                        A_sb[:, 2 * TOFF[c]:2 * TOFF[c] + 2 * TILES_C],
                        AluOpType.add)
                    E = dE_pool.tile([128, 2 * max(TILES)], BF16, tag="E")
                    nc.scalar.activation(E[:, 0:2 * TILES_C],
                                         d[:, 0:2 * TILES_C], AF.Exp)
                    Z1 = sm_pool.tile([128, 16], F32, tag="Z1")
                    nc.vector.tensor_reduce(
                        Z1[:].rearrange("p (s a) -> p s a", a=2),
                        E[:, 0:2 * TILES_C].rearrange(
                            "p (s r a) -> p s a r", s=CH_SEGS, r=TPS, a=2),
                        axis=AX, op=AluOpType.add)
                    ctx["E"], ctx["Z1"] = E, Z1
                elif stage == 1:
                    st = stpsum.tile([128, 160], F32, tag="st")
                    nc.tensor.matmul(st[0:16, 0:128],
                                     lhsT=ctx["Z1"][:],
                                     rhs=IDNF_sb[:], is_transpose=True,
                                     start=True, stop=True)
                    zcol = sm_pool.tile([16, 1], F32, tag="zcol")
                    nc.vector.tensor_reduce(zcol[:], st[0:16, 0:128],
                                            axis=AX, op=AluOpType.add)
                    nc.vector.tensor_scalar_add(zcol[:], zcol[:], 1.0e-8)
                    zinv = sm_pool.tile([16, 1], F32, tag="zinv")
                    nc.vector.reciprocal(zinv[:], zcol[:])
                    sc = sm_pool.tile([16, 1], F32, tag="sc")
                    nc.vector.tensor_tensor(sc[:], zinv[:],
                                            RSN_sb[:, c:c + 1],
                                            AluOpType.mult)
                    ctx["st"], ctx["sc"] = st, sc
                elif stage == 2:
                    st = ctx["st"]
                    nc.tensor.matmul(st[0:1, 144:160],
                                     lhsT=ctx["sc"][:],
                                     rhs=IDNF_sb[0:16, 0:16],
                                     is_transpose=True,
                                     start=True, stop=True)
                    scrow = sm_pool.tile([1, 16], F32, tag="scrow")
                    nc.vector.tensor_copy(scrow[:], st[0:1, 144:160])
                    nc.tensor.matmul(st[:, 128:144],
                                     lhsT=ONES_sb[:],
                                     rhs=scrow[:],
                                     start=True, stop=True)
                elif stage == 3:
                    st = ctx["st"]
                    screp = st[:, 128:144].rearrange(
                        "p (s a) -> p s a", a=2).unsqueeze(2) \
                        .broadcast_to([128, CH_SEGS, TPS, 2])
                    Ew = dE_pool.tile([128, 2 * max(TILES)], BF16, tag="Ew")
                    nc.vector.tensor_tensor(
                        Ew[:, 0:2 * TILES_C].rearrange(
                            "p (s r a) -> p s r a", s=CH_SEGS, r=TPS, a=2),
                        ctx["E"][:, 0:2 * TILES_C].rearrange(
                            "p (s r a) -> p s r a", s=CH_SEGS, r=TPS, a=2),
                        screp, AluOpType.mult)
                    ctx["Ew"] = Ew

            def pool_back(c, nat, ctx):
                """Pooling matmuls + scatter for chunk c."""
                TPS = tps_list[c]
                Ew = ctx["Ew"]
                pp = ppsum.tile([128, 32], F32, tag="pp")
                for s in range(CH_SEGS):
                    for hh in range(2):
                        for r in range(TPS):
                            t = s * TPS + r
                            nc.tensor.matmul(
                                pp[:, hh * 16 + 2 * s: hh * 16 + 2 * s + 2],
                                lhsT=nat[:, t * H + 128 * hh:
                                         t * H + 128 * hh + 128],
                                rhs=Ew[:, 2 * t: 2 * t + 2],
                                start=(r == 0), stop=(r == TPS - 1))
                for hh in range(2):
                    grp = pp[:, hh * 16:(hh + 1) * 16].rearrange(
                        "p (s a) -> p a s", a=2)
                    for a in range(2):
                        nc.vector.tensor_copy(
                            pTsb[:, (a * 2 + hh) * SEGS + c * CH_SEGS:
                                 (a * 2 + hh) * SEGS + (c + 1) * CH_SEGS],
                            grp[:, a])

            # stats stage to run after L1 group hb of the NEXT chunk
            STAGE_AT = {0: 0, 2: 1, 4: 2, 5: 3}

            def front(c, prev):
                """DMA + L1/tanh + L2 for chunk c, with chunk c-1's stats
                chain interleaved between L1 groups (keeps PE fed)."""
                TPS = tps_list[c]
                TILES_C = TILES[c]
                NODES_C = NODES[c]
                NGRP = NODES_C // 512
                NHB = 2 * TPS  # [128,1024] tanh tiles ( = 2*NODES_C/1024 )

                xT8 = xT_pool.tile([128, XT_MAX], FP8, tag="xT8")
                if c == 0:
                    # split so L1 can start on the first half-chunk
                    hn = (NODES_C // 2) // 512 * 512
                    for k in range(2):
                        nc.sync.dma_start(
                            xT8[:, k * NODES_C:k * NODES_C + hn],
                            xt_d[:, 2 * NOFF[c] + k * NODES_C:
                                 2 * NOFF[c] + k * NODES_C + hn])
                    for k in range(2):
                        nc.sync.dma_start(
                            xT8[:, k * NODES_C + hn:(k + 1) * NODES_C],
                            xt_d[:, 2 * NOFF[c] + k * NODES_C + hn:
                                 2 * NOFF[c] + (k + 1) * NODES_C])
                else:
                    nc.sync.dma_start(
                        xT8[:, 0:2 * NODES_C],
                        xt_d[:, 2 * NOFF[c]:2 * NOFF[c] + 2 * NODES_C])
                nat = nat_pool.tile([128, NAT_MAX], BF16, tag="nat")
                nc.sync.dma_start(nat[:, 0:TILES_C * H],
                                  x_d[:, TOFF[c] * H:(TOFF[c] + TILES_C) * H])

                # L1 + tanh; hidden stream col = a*NODES_C + node_local
                hsb = h_pool.tile([128, XT_MAX], BF16, tag="hsb")
                for hb in range(NHB):
                    a = hb // TPS
                    hp = hpsum.tile([128, 1024], F32, tag="hp")
                    for h2 in range(2):
                        g = (hb * 2 + h2) % NGRP
                        for k in range(2):
                            nc.tensor.matmul(
                                hp[:, h2 * 512:(h2 + 1) * 512],
                                lhsT=W1_sb[:, (a * 2 + k) * 128:
                                           (a * 2 + k + 1) * 128],
                                rhs=xT8[:, k * NODES_C + g * 512:
                                        k * NODES_C + (g + 1) * 512],
                                start=(k == 0), stop=(k == 1))
                    nc.scalar.activation(
                        hsb[:, hb * 1024:(hb + 1) * 1024], hp[:],
                        AF.Tanh, bias=B1_sb[:, a:a + 1],
                        scale=1.0 / (XSC * WSC))
                    if prev is not None and hb in STAGE_AT:
                        stats_stage(STAGE_AT[hb], prev[0], prev[2], prev[3])

                # L2: s_ps[node, (t,a)]
                s_ps = spsum.tile([128, 2 * max(TILES)], F32, tag="s")
                for t in range(TILES_C):
                    for a in range(2):
                        nc.tensor.matmul(
                            s_ps[:, 2 * t: 2 * t + 2],
                            lhsT=hsb[:, a * NODES_C + t * 128:
                                     a * NODES_C + (t + 1) * 128],
                            rhs=GL2_sb[:, 2 * a: 2 * a + 2],
                            start=(a == 0), stop=(a == 1))
                if prev is not None:
                    pool_back(prev[0], prev[1], prev[3])
                return nat, s_ps

            prev = None
            for c in range(NCHUNK):
                nat, s_ps = front(c, prev)
                prev = (c, nat, s_ps, {})
                if c == 1:
                    # out-proj weights: late enough to keep the head clear,
                    # early enough to be resident before the tail
                    nc.sync.dma_start(OW_sb[:], ow_d[:])
                    nc.sync.dma_start(OB_sb[:], ob_d[:])
            for stage in range(4):
                stats_stage(stage, prev[0], prev[2], prev[3])
            pool_back(prev[0], prev[1], prev[3])

            # ---- output projection ----
            yps = hpsum.tile([128, H], F32, tag="hp")
            for f2b in range(4):
                nc.tensor.matmul(yps[0:SEGS, :],
                                 lhsT=pTsb[:, f2b * SEGS:(f2b + 1) * SEGS],
                                 rhs=OW_sb[:, f2b * H:(f2b + 1) * H],
                                 start=(f2b == 0), stop=False)
            nc.tensor.matmul(yps[0:SEGS, :],
                             lhsT=ONES_sb[:, 0:SEGS],
                             rhs=OB_sb[:],
                             start=False, stop=True)
            ysb = acc_pool.tile([SEGS, H], F32, tag="y")
            nc.scalar.copy(ysb[:], yps[0:SEGS, :])
            nc.sync.dma_start(y_d[:], ysb[:])

    nc.compile()
    return nc


def _plan(batch, B):
    """Segment geometry: rank-sorted round-robin core assignment."""
    offs = np.searchsorted(batch, np.arange(B + 1))
    lens = np.diff(offs)
    order = np.argsort(lens, kind="stable")          # rank -> segment
    segs_per_core = B // N_CORES
    nchunk = segs_per_core // CH_SEGS
    sl = lens[order]
    tps_list = []
    for j in range(nchunk):
        mx = int(sl[j * CH_SEGS * N_CORES:(j + 1) * CH_SEGS * N_CORES].max())
        tps_list.append(max(int(math.ceil(mx / 128.0)), 1))
    return offs, lens, order, tuple(tps_list), segs_per_core


def _host_prep(x, batch, lysine_mask, gw1, gb1, gw2, gb2,
               lw1, lb1, lw2, lb2, ow, ob, B=1024, n_cores=N_CORES):
    """Build per-core input maps. Returns (in_maps, key, segs_per_core, B)."""
    N = x.shape[0]
    batch = np.asarray(batch).astype(np.int64)
    offs, lens, order, tps_list, segs_per_core = _plan(batch, B)
    NCHUNK = len(tps_list)
    TILES = [CH_SEGS * t for t in tps_list]
    TOFF = np.concatenate([[0], np.cumsum(TILES)]).astype(np.int64)
    TT = int(TOFF[-1])
    NODES = [128 * t for t in TILES]
    NOFF = (128 * TOFF).astype(np.int64)
    CN = 128 * TT

    x = np.asarray(x, dtype=np.float32)
    lys = np.asarray(lysine_mask).astype(bool)

    # per-segment destination bases in the global padded [8*CN] node space
    rank_of = np.empty(B, dtype=np.int64)
    rank_of[order] = np.arange(B)
    core_of = rank_of % n_cores
    slot_of = rank_of // n_cores
    chunk_of = slot_of // CH_SEGS
    sloc_of = slot_of % CH_SEGS
    tpss = np.asarray(tps_list, dtype=np.int64)
    seg_base = (core_of * CN + NOFF[chunk_of]
                + sloc_of * 128 * tpss[chunk_of])
    pos = np.arange(N) - offs[batch]
    dest = seg_base[batch] + pos

    x_all = np.zeros((n_cores * CN, H), dtype=ml_dtypes.bfloat16)
    x_all[dest] = x
    x8_all = np.zeros((n_cores * CN, H), dtype=ml_dtypes.float8_e3m4)
    x8_all[dest] = (x * XSC).astype(ml_dtypes.float8_e3m4)
    a_all = np.full((n_cores * CN, 2), NEG, dtype=np.float32)
    a_all[dest, 0] = float(gb2[0])
    a_all[dest, 1] = np.where(lys, float(lb2[0]), NEG)

    # per-(core,slot) 1/sqrt(len)
    rsn_cs = np.empty((n_cores, segs_per_core), dtype=np.float32)
    rsn_cs[core_of, slot_of] = 1.0 / np.sqrt(
        np.maximum(lens, 1).astype(np.float32))

    # weights (shared)
    w1 = np.concatenate([gw1[:128], gw1[128:], lw1[:128], lw1[128:]], axis=1)
    w18 = (w1 * WSC).astype(ml_dtypes.float8_e3m4)  # [128, 512]
    b1 = np.stack([gb1, lb1], axis=1).astype(np.float32)  # [128, 2]
    z = np.zeros_like(gw2)
    gl2p = np.concatenate([gw2, z, z, lw2], axis=1).astype(
        ml_dtypes.bfloat16)  # [128, 4]
    ow_blocks = np.concatenate(
        [ow[0:128], ow[128:256], ow[256:384], ow[384:512]],
        axis=1).astype(np.float32)  # [128, 1024]
    ob_r = np.asarray(ob, dtype=np.float32).reshape(1, H)
    ones = np.ones((1, 128), dtype=np.float32)
    idnf = np.eye(128, dtype=np.float32)

    in_maps = []
    for core in range(n_cores):
        xc_flat = np.empty((128, TT * H), dtype=ml_dtypes.bfloat16)
        x8_flat = np.empty((128, 2 * CN), dtype=ml_dtypes.float8_e3m4)
        a_flat = np.empty((128, 2 * TT), dtype=np.float32)
        for c in range(NCHUNK):
            blk = x_all[core * CN + NOFF[c]: core * CN + NOFF[c] + NODES[c]]
            xc = blk.reshape(TILES[c], 128, H).transpose(1, 0, 2)
            xc_flat[:, TOFF[c] * H:(TOFF[c] + TILES[c]) * H] = \
                xc.reshape(128, TILES[c] * H)
            b8 = x8_all[core * CN + NOFF[c]: core * CN + NOFF[c] + NODES[c]]
            x8 = b8.reshape(NODES[c], 2, 128).transpose(2, 1, 0)
            x8_flat[:, 2 * NOFF[c]:2 * NOFF[c] + 2 * NODES[c]] = \
                x8.reshape(128, 2 * NODES[c])
            ab = a_all[core * CN + NOFF[c]: core * CN + NOFF[c] + NODES[c]]
            ac = ab.reshape(TILES[c], 128, 2).transpose(1, 0, 2)
            a_flat[:, 2 * TOFF[c]:2 * TOFF[c] + 2 * TILES[c]] = \
                ac.reshape(128, 2 * TILES[c])
        rs = rsn_cs[core].reshape(NCHUNK, CH_SEGS)
        rsc = np.ascontiguousarray(
            np.repeat(rs.T[:, None, :], 2, axis=1).reshape(
                2 * CH_SEGS, NCHUNK)).astype(np.float32)
        in_maps.append({
            "x": xc_flat, "xt8": x8_flat, "amask": a_flat, "rsn": rsc,
            "w1": w18, "b1": b1, "gl2": gl2p, "ow": ow_blocks, "ob": ob_r,
            "ones": ones, "idnf": idnf,
        })
    return in_maps, tps_list, segs_per_core, B


def kernel(**inputs):
    x = np.asarray(inputs["x"])
    batch = np.asarray(inputs["batch"]).astype(np.int64)
    B = 1024
    in_maps, tps_list, segs_per_core, B = _host_prep(
        x, batch, inputs["lysine_mask"],
        np.asarray(inputs["gw1"], np.float32), np.asarray(inputs["gb1"], np.float32),
        np.asarray(inputs["gw2"], np.float32), np.asarray(inputs["gb2"], np.float32),
        np.asarray(inputs["lw1"], np.float32), np.asarray(inputs["lb1"], np.float32),
        np.asarray(inputs["lw2"], np.float32), np.asarray(inputs["lb2"], np.float32),
        np.asarray(inputs["ow"], np.float32), np.asarray(inputs["ob"], np.float32))

    key = (tps_list, segs_per_core)
    if key not in _cache:
        _cache[key] = _build(*key)
    nc = _cache[key]

    res = run_bass_kernel_spmd(nc, in_maps, core_ids=list(range(N_CORES)))

    # un-permute: core c slot i holds segment order[i*8 + c]
    _, _, order, _, _ = _plan(batch, B)
    out = np.empty((B, H), dtype=np.float32)
    for c in range(N_CORES):
        segs = order[np.arange(segs_per_core) * N_CORES + c]
        out[segs] = res.results[c]["y"]
    return out
